# revision 27
# baseline (speedup 1.0000x reference)
"""AdaptiveJacobianPrunedViT on 8 TRN2 NeuronCores (Bass/Tile, SPMD data-parallel).

Sharding: pure data parallel over batch (8 images/core). Token pruning uses
batch-mean importance; local importance vectors are AllReduce-summed across
cores before top-k so every core keeps the identical (reference) token set.

Per-core layouts:
  t (residual):  token-major flat [8*M, 768] as [128,768] f32r tiles
  xn/q/k/o:      feature-major [768, 8*M] as 6 [128, R] f32r tiles
  v:             token-major per-(b,chunk) [<=128, 768] tiles (PV lhsT)
All GEMMs run fp32r (11-bit RNE input rounding, fp32 accumulate).
Importance comparisons/broadcasts use exact fp32 matmuls.
"""
import numpy as np

MIN_TOKENS, KEEP_FRAC = 16, 0.85
H, HD, D, FF, L = 12, 64, 768, 3072, 12
B_LOC, N_CORES, LN_EPS, V_OUT = 8, 8, 1e-6, 1000
GRID, PATCH = 14, 16


def schedule():
    sched, M = [], GRID * GRID + 1
    for _ in range(L):
        Np = M - 1
        nxt = max(MIN_TOKENS, int(Np * KEEP_FRAC)) if Np > MIN_TOKENS else Np
        if Np > MIN_TOKENS and nxt < Np:
            sched.append((M, nxt + 1, nxt))
            M = nxt + 1
        else:
            sched.append((M, M, None))
    return sched


SCHED = schedule()


def round_fp32r(a):
    a = np.ascontiguousarray(a, dtype=np.float32)
    u = a.view(np.uint32).astype(np.uint64)
    sh = 12
    half, mask = np.uint64(1 << (sh - 1)), np.uint64((1 << sh) - 1)
    frac = u & mask
    u2 = u & ~mask
    up = (frac > half) | ((frac == half) & (((u >> np.uint64(sh)) & np.uint64(1)) == 1))
    u2 = u2 + np.where(up, np.uint64(1 << sh), np.uint64(0))
    return u2.astype(np.uint32).view(np.float32)


def cdiv(a, b):
    return (a + b - 1) // b


def chunks(total, step, start=0):
    return [(s, min(step, total - s)) for s in range(start, total, step)]


def build_graph():
    import concourse.bacc as bacc
    import concourse.mybir as mybir
    import concourse.tile as tile
    import contextlib

    F32, F32R = mybir.dt.float32, mybir.dt.float32r
    AF, OP = mybir.ActivationFunctionType, mybir.AluOpType

    M0 = SCHED[0][0]
    R0 = B_LOC * M0          # 1576
    NT0 = cdiv(R0, 128)      # 13
    SC = float(HD) ** -0.5

    nc = bacc.Bacc("TRN2", target_bir_lowering=False, debug=False, num_devices=N_CORES)

    xtok_d = nc.dram_tensor("xtok_fm", [D, R0], F32R, kind="ExternalInput")
    posrep_d = nc.dram_tensor("posrep", [R0, D], F32, kind="ExternalInput")
    pw_d = nc.dram_tensor("pw", [D, D], F32R, kind="ExternalInput")
    wq_d = nc.dram_tensor("wq", [L, D, D], F32R, kind="ExternalInput")
    wk_d = nc.dram_tensor("wk", [L, D, D], F32R, kind="ExternalInput")
    wv_d = nc.dram_tensor("wv", [L, D, D], F32R, kind="ExternalInput")
    wo_d = nc.dram_tensor("wo", [L, D, D], F32R, kind="ExternalInput")
    w1_d = nc.dram_tensor("w1", [L, D, FF], F32R, kind="ExternalInput")
    w2_d = nc.dram_tensor("w2", [L, FF, D], F32R, kind="ExternalInput")
    hw_d = nc.dram_tensor("hw", [D, V_OUT], F32R, kind="ExternalInput")
    iota_d = nc.dram_tensor("iota128f", [128, 128], F32, kind="ExternalInput")
    pcol_d = nc.dram_tensor("pcol128", [128, 1], F32, kind="ExternalInput")
    ident_d = nc.dram_tensor("ident128", [128, 128], F32R, kind="ExternalInput")
    e1r_d = nc.dram_tensor("e1r128", [128, 2], F32, kind="ExternalInput")
    onesf_d = nc.dram_tensor("onesf128", [128, 2], F32, kind="ExternalInput")
    onesr1_d = nc.dram_tensor("onesr_1x", [1, 128], F32R, kind="ExternalInput")
    ones1_d = nc.dram_tensor("ones_1x", [1, 128], F32, kind="ExternalInput")
    hmaskr_d = nc.dram_tensor("headmaskr", [D, 128], F32R, kind="ExternalInput")
    ones2dr_d = nc.dram_tensor("ones2dr", [128, 128], F32R, kind="ExternalInput")

    out_d = nc.dram_tensor("out", [B_LOC, V_OUT], F32, kind="ExternalOutput")

    with tile.TileContext(nc) as tc, contextlib.ExitStack() as ctx:
        ctx.enter_context(nc.allow_low_precision(reason="fp32r compute by design"))
        sb = ctx.enter_context(tc.tile_pool(name="sb", bufs=1))
        ps = ctx.enter_context(tc.tile_pool(name="ps", bufs=1, space="PSUM"))
        dram = ctx.enter_context(tc.tile_pool(name="dram", bufs=1, space="DRAM"))

        _uid = [0]

        def uid():
            _uid[0] += 1
            return _uid[0]

        def psA(p, f, name=None):
            return ps.tile([p, f], F32, tag="psA", name=f"pA{uid()}", bufs=2,
                           padded_shape=[128, 512])

        def psQ(p, f):
            return ps.tile([p, f], F32, tag="psQ", name=f"pQ{uid()}", bufs=1,
                           padded_shape=[128, 512])

        def psI(p, f):
            return ps.tile([p, f], F32, tag="psI", name=f"pI{uid()}", bufs=1,
                           padded_shape=[128, 512])

        def psB(p, f, name=None):
            return ps.tile([p, f], F32, tag="psB", name=f"pB{uid()}", bufs=2,
                           padded_shape=[128, 1024])

        # size-classed SBUF scratch: clsA = [128, <=864] f32r-ish slots
        def clsA(p, f, dtype, nbufs=8):
            return sb.tile([p, f], dtype, tag="clsA", name=f"cA{uid()}", bufs=nbufs,
                           padded_shape=[128, 768])

        def clsB(p, f, dtype, nbufs=18):
            return sb.tile([p, f], dtype, tag="clsB", name=f"cB{uid()}", bufs=nbufs,
                           padded_shape=[128, 640])

        # ---- consts
        iota_t = sb.tile([128, 128], F32, name="iota_t")
        pcol_t = sb.tile([128, 1], F32, name="pcol_t")
        nc.sync.dma_start(pcol_t[:], pcol_d[:, :])
        ident_t = sb.tile([128, 128], F32R, name="ident_t")
        e1r_t = sb.tile([128, 2], F32, name="e1r_t")
        onesf_t = sb.tile([128, 2], F32, name="onesf_t")
        onesr1_t = sb.tile([1, 128], F32R, name="onesr1_t")
        ones1_t = sb.tile([1, 128], F32, name="ones1_t")
        hmaskr_t = [sb.tile([128, 128], F32R, name=f"hmr{i}") for i in range(6)]
        ones2dr_t = sb.tile([128, 128], F32R, name="ones2dr_t")
        nc.sync.dma_start(ones2dr_t[:], ones2dr_d[:, :])
        nc.sync.dma_start(iota_t[:], iota_d[:, :])
        nc.sync.dma_start(ident_t[:], ident_d[:, :])
        nc.sync.dma_start(e1r_t[:], e1r_d[:, :])
        nc.sync.dma_start(onesf_t[:], onesf_d[:, :])
        nc.sync.dma_start(onesr1_t[:], onesr1_d[:, :])
        nc.sync.dma_start(ones1_t[:], ones1_d[:, :])
        for i in range(6):
            nc.sync.dma_start(hmaskr_t[i][:], hmaskr_d[128 * i:128 * (i + 1), :])
        eps_t = sb.tile([128, 1], F32, name="eps_t")
        nc.vector.memset(eps_t[:], LN_EPS)

        def rows_of(i, R):
            return min(128, R - 128 * i)

        # persistent residual tiles (13 x [128, 768] f32r)
        t_t = [sb.tile([128, D], F32R, tag=f"t{i}", name=f"t{i}") for i in range(NT0)]

        # weight slot: half-matrices [128, 3*768] (wq/wk/wv/wo halves), bufs=2
        def load_whalf(dram_ap, tag_l, half):
            t = sb.tile([128, 3 * D], F32R, tag="wslot", name=f"w{tag_l}_{half}_{uid()}",
                        bufs=6, padded_shape=[128, 3 * D])
            nc.sync.dma_start(
                t[:].rearrange("p (k n) -> p k n", k=3),
                dram_ap[128 * 3 * half:128 * 3 * (half + 1), :]
                .rearrange("(k p) n -> p k n", p=128))
            return t

        def ln_stats(tiles, R, nt_max):
            NT = cdiv(R, 128)
            sums = sb.tile([128, 2 * NT0], F32, tag="lnsums", name=f"sums{uid()}")
            junk = clsA(128, D, F32)
            for i in range(NT):
                r = rows_of(i, R)
                nc.scalar.activation(junk[:r, :], tiles[i][:r, :], AF.Identity,
                                     accum_out=sums[:r, i:i + 1])
                nc.scalar.activation(junk[:r, :], tiles[i][:r, :], AF.Square,
                                     accum_out=sums[:r, NT0 + i:NT0 + i + 1])
            stats = sb.tile([128, 4 * NT0], F32, tag="lnstats", name=f"stats{uid()}")
            mu = stats[:, 0:NT]
            rstd = stats[:, NT0:NT0 + NT]
            nmurs = stats[:, 2 * NT0:2 * NT0 + NT]
            scr = stats[:, 3 * NT0:3 * NT0 + NT]
            nc.vector.tensor_scalar(mu, sums[:, 0:NT], 1.0 / D, None, OP.mult)
            nc.vector.tensor_scalar(scr, sums[:, NT0:NT0 + NT], 1.0 / D, None, OP.mult)
            nc.vector.tensor_tensor(rstd, mu, mu, OP.mult)
            nc.vector.tensor_tensor(scr, scr, rstd, OP.subtract)      # var
            nc.scalar.activation(scr, scr, AF.Sqrt, bias=eps_t[:, :])  # std
            nc.vector.reciprocal(rstd, scr)
            nc.vector.tensor_tensor(nmurs, mu, rstd, OP.mult)
            nc.vector.tensor_scalar(nmurs, nmurs, -1.0, None, OP.mult)
            return stats

        def xn_cols(stats, c0, cw, R):
            """LN-applied xn feature-major covering cols [c0, c0+cw).
            Works on the 128-aligned covering range; returns (tiles, base)."""
            NT = cdiv(R, 128)
            rstd = stats[:, NT0:NT0 + NT]
            nmurs = stats[:, 2 * NT0:2 * NT0 + NT]
            t0a = (c0 // 128) * 128
            t1a = min(cdiv(c0 + cw, 128) * 128, ((R + 127) // 128) * 128)
            tis = list(range(t0a // 128, t1a // 128))
            cwa = sum(rows_of(ti, R) for ti in tis)
            dst = [clsB(128, cwa, F32R) for _ in range(6)]
            stage_list = []
            dl = 0
            for ti in tis:
                take = rows_of(ti, R)
                stage = clsA(128, D, F32R)
                nc.scalar.activation(stage[:take, :], t_t[ti][:take, :],
                                     AF.Identity,
                                     scale=rstd[:take, ti:ti + 1],
                                     bias=nmurs[:take, ti:ti + 1])
                stage_list.append((stage, take, dl))
                dl += take
            for kt in range(6):
                pt = psB(128, cwa)
                for j, (stage, take, dl) in enumerate(stage_list):
                    nc.tensor.matmul(
                        pt[:, dl:dl + take].bitcast(F32R),
                        stage[:take, 128 * kt:128 * (kt + 1)],
                        ident_t[:take, :take], is_transpose=True,
                        start=(j == 0), stop=(j == len(stage_list) - 1))
                nc.vector.tensor_copy(dst[kt][:, :cwa], pt[:, :cwa].bitcast(F32R))
            return dst, t0a

        def pair_cols(M):
            """[(w0, wlen, voffs per b in pair)] covering batches in pairs."""
            out = []
            for b0 in range(0, B_LOC, 2):
                out.append((b0 * M, 2 * M, b0))
            return out

        # =========================================================
        # patch embed: t = xtok @ pw + posrep
        # =========================================================
        pwh = [load_whalf(pw_d[:, :], "pw", h) for h in range(2)]
        for i in range(NT0):
            r = rows_of(i, R0)
            xtk = [clsB(128, 128, F32R) for _ in range(6)]
            for kt in range(6):
                nc.sync.dma_start(xtk[kt][:, :r],
                                  xtok_d[128 * kt:128 * (kt + 1), 128 * i:128 * i + r])
            pos_t = clsA(128, D, F32)
            nc.sync.dma_start(pos_t[:r, :], posrep_d[128 * i:128 * i + r, :])
            pt = psB(128, D)
            for n0, nw in chunks(D, 512):
                for kt in range(6):
                    half, k3 = kt // 3, kt % 3
                    nc.tensor.matmul(
                        pt[:, n0:n0 + nw],
                        xtk[kt][:, :],
                        pwh[half][:, D * k3 + n0:D * k3 + n0 + nw],
                        start=(kt == 0), stop=(kt == 5))
            nc.vector.tensor_tensor(t_t[i][:r, :], pt[:r, :], pos_t[:r, :], OP.add)

        # =========================================================
        # transformer layers
        # =========================================================
        o_dram = dram.tile([D, R0], F32, tag="odram", name="o_dram")
        for l in range(L):
            M, Mq, n_next = SCHED[l]
            R, Rq = B_LOC * M, B_LOC * Mq
            NT, NTq = cdiv(R, 128), cdiv(Rq, 128)

            st1 = ln_stats(t_t, R, NT)
            wkh = [load_whalf(wk_d[l], f"k{l}", h) for h in range(2)]
            wvh = [load_whalf(wv_d[l], f"v{l}", h) for h in range(2)]
            wqh = [load_whalf(wq_d[l], f"q{l}", h) for h in range(2)]

            den = sb.tile([12, 16], F32, tag="den", name=f"den{l}")
            qc_sb = sb.tile([128, 12], F32, tag="qc", name=f"qc{l}", bufs=1)
            qcbd = sb.tile([128, 128], F32R, tag="qcbd", name=f"qcbd{l}", bufs=2)
            imp_ps = psI(1, M)
            for pi, (w0, wl, b0) in enumerate(pair_cols(M)):
                xnp, xb = xn_cols(st1, w0, wl, R)
                xo = w0 - xb
                qc_ps = psQ(128, 12)
                for nt in range(6):
                    for kt in range(6):
                        half, k3 = kt // 3, kt % 3
                        nc.tensor.matmul(
                            qc_ps[:, 2 * nt:2 * nt + 2],
                            wqh[half][:, D * k3 + 128 * nt:D * k3 + 128 * (nt + 1)],
                            xnp[kt][:, xo:xo + wl:M],
                            start=(nt == 0 and kt == 0), stop=(nt == 5 and kt == 5))
                nc.scalar.activation(qc_sb[:], qc_ps[:], AF.Copy)
                kpre = [clsB(128, wl, F32R) for _ in range(6)]
                vpre = [clsB(128, wl, F32R) for _ in range(6)]
                for dst, wh in ((kpre, wkh), (vpre, wvh)):
                    for nt in range(6):
                        pt = psA(128, wl)
                        for kt in range(6):
                            h2, k32 = kt // 3, kt % 3
                            nc.tensor.matmul(
                                pt[:, :wl],
                                wh[h2][:, D * k32 + 128 * nt:D * k32 + 128 * (nt + 1)],
                                xnp[kt][:, xo:xo + wl],
                                start=(kt == 0), stop=(kt == 5))
                        nc.scalar.activation(dst[nt][:, :wl], pt[:, :wl], AF.Copy)
                # vnorm for the pair
                vp = psA(128, wl)
                sqv = clsB(128, wl, F32R)
                for kt in range(6):
                    nc.scalar.activation(sqv[:, :wl], vpre[kt][:, :wl], AF.Square)
                    nc.tensor.matmul(vp[:, :wl], hmaskr_t[kt][:], sqv[:, :wl],
                                     start=(kt == 0), stop=(kt == 5))
                vnp = clsB(12, wl, F32)
                nc.scalar.activation(vnp[:12, :wl], vp[:12, :wl], AF.Sqrt)
                # CLS attention -> importance contribution
                pclsp = clsB(12, wl, F32)
                for bi, b in enumerate((b0, b0 + 1)):
                    sc = psA(128, wl)
                    for kt in range(6):
                        nc.vector.tensor_scalar(qcbd[:], hmaskr_t[kt][:].bitcast(F32),
                                                qc_sb[:, 2 * kt + bi:2 * kt + bi + 1],
                                                None, OP.mult)
                        nc.tensor.matmul(sc[:, :wl], qcbd[:], kpre[kt][:, :wl],
                                         start=(kt == 0), stop=(kt == 5))
                    voff = bi * M
                    nc.scalar.activation(pclsp[:12, voff:voff + M],
                                         sc[:12, voff:voff + M], AF.Exp, scale=SC)
                    nc.vector.tensor_reduce(den[:, b:b + 1],
                                            pclsp[:12, voff:voff + M],
                                            mybir.AxisListType.X, OP.add)
                nc.vector.reciprocal(den[:, 8 + b0:10 + b0], den[:, b0:b0 + 2])
                for bi, b in enumerate((b0, b0 + 1)):
                    voff = bi * M
                    nc.vector.tensor_scalar(vnp[:12, voff:voff + M],
                                            vnp[:12, voff:voff + M],
                                            den[:, 8 + b:9 + b], None, OP.mult)
                nc.vector.tensor_tensor(pclsp[:12, :wl], pclsp[:12, :wl],
                                        vnp[:12, :wl], OP.mult)
                for bi, b in enumerate((b0, b0 + 1)):
                    nc.tensor.matmul(imp_ps[:, :], onesf_t[:12, 0:1],
                                     pclsp[:12, bi * M:bi * M + M],
                                     start=(b == 0), stop=(b == B_LOC - 1))
            # ---- AllReduce importance
            imp_sb = sb.tile([1, 200], F32, tag="imp", name=f"imp{l}")
            nc.vector.tensor_copy(imp_sb[:, :M], imp_ps[:, :])
            bin_t = dram.tile([1, M], F32, tag="arin", name=f"arin{l}")
            bout_t = dram.tile([1, M], F32, tag="arout", name=f"arout{l}",
                               addr_space="Shared")
            nc.sync.dma_start(bin_t[:], imp_sb[:, :M])
            nc.gpsimd.collective_compute(
                "AllReduce", OP.add, replica_groups=[list(range(N_CORES))],
                ins=[bin_t.opt()], outs=[bout_t.opt()])
            impg = sb.tile([1, 200], F32, tag="impg", name=f"impg{l}")
            nc.sync.dma_start(impg[:, :M], bout_t[:])

            # ---- ranks -> mask -> pos
            KC = cdiv(M, 128)
            impcol = sb.tile([128, 2], F32, tag="impcol", name=f"impcol{l}")
            for kc in range(KC):
                cnt = rows_of(kc, M)
                icp = psA(128, 1)
                nc.tensor.matmul(icp[:cnt, :], impg[:, 128 * kc:128 * kc + cnt],
                                 ones1_t[:, 0:1], start=True, stop=True)
                nc.vector.tensor_copy(impcol[:cnt, kc:kc + 1], icp[:cnt, :])
            rank_ps = psI(1, M)
            for kc in range(KC):
                cnt = rows_of(kc, M)
                ibc = psA(128, M)
                nc.tensor.matmul(ibc[:cnt, :], ones1_t[:, :cnt], impg[:, :M],
                                 start=True, stop=True)
                Ct = clsB(128, M, F32)
                eqt = clsB(128, M, F32)
                C2t = clsB(128, M, F32)
                nc.vector.tensor_scalar(Ct[:cnt, :M], ibc[:cnt, :M],
                                        impcol[:cnt, kc:kc + 1], None, OP.is_lt)
                nc.vector.tensor_scalar(eqt[:cnt, :M], ibc[:cnt, :M],
                                        impcol[:cnt, kc:kc + 1], None, OP.is_equal)
                # C2t mask: (global col j) > (global row k = 128*kc + p)
                pshk = sb.tile([128, 1], F32, tag="possh", name=f"pk{uid()}", bufs=4)
                for j0, jw in chunks(M, 128):
                    nc.vector.tensor_scalar(pshk[:cnt, :], pcol_t[:cnt, :],
                                            float(128 * kc - j0), None, OP.add)
                    nc.vector.tensor_scalar(C2t[:cnt, j0:j0 + jw],
                                            iota_t[:cnt, :jw], pshk[:cnt, :],
                                            None, OP.is_gt)
                nc.vector.tensor_tensor(C2t[:cnt, :M], eqt[:cnt, :M],
                                        C2t[:cnt, :M], OP.mult)
                lhs = e1r_t if kc == 0 else onesf_t
                nc.tensor.matmul(rank_ps[:, :], lhs[:cnt, 0:1], Ct[:cnt, :M],
                                 start=(kc == 0), stop=False)
                nc.tensor.matmul(rank_ps[:, :], lhs[:cnt, 0:1], C2t[:cnt, :M],
                                 start=False, stop=(kc == KC - 1))
            mask = sb.tile([1, 200], F32R, tag="mask", name=f"mask{l}")
            nc.vector.tensor_scalar(mask[:, :M], rank_ps[:, :], float(n_next), None,
                                    OP.is_lt)
            nc.vector.tensor_copy(mask[:, 0:1], onesf_t[0:1, 0:1])
            mflat = sb.tile([1, R0], F32R, tag="mflat", name=f"mflat{l}")
            for b in range(B_LOC):
                nc.vector.tensor_copy(mflat[:, b * M:(b + 1) * M], mask[:, :M])
            cum = sb.tile([1, R0], F32, tag="cum", name=f"cum{l}")
            nc.vector.tensor_tensor_scan(cum[:, :R], mflat[:, :R].bitcast(F32),
                                         mflat[:, :R].bitcast(F32), 0.0,
                                         OP.add, OP.max)
            nc.vector.tensor_tensor(cum[:, :R], cum[:, :R], mflat[:, :R].bitcast(F32),
                                    OP.subtract)  # exclusive positions, in place
            poscol = sb.tile([128, 2 * NT0], F32, tag="poscol", name=f"poscol{l}")
            for rc in range(NT):
                cnt = rows_of(rc, R)
                pcp = psA(128, 2)
                nc.tensor.matmul(pcp[:cnt, 0:1], cum[:, 128 * rc:128 * rc + cnt],
                                 ones1_t[:, 0:1], start=True, stop=False)
                nc.tensor.matmul(pcp[:cnt, 1:2],
                                 mflat[:, 128 * rc:128 * rc + cnt].bitcast(F32),
                                 ones1_t[:, 0:1], start=False, stop=True)
                nc.vector.tensor_copy(poscol[:cnt, 2 * rc:2 * rc + 2], pcp[:cnt, :])

            def win(rc):
                lo, hi = None, None
                for rr in range(128 * rc, min(128 * rc + 128, R)):
                    b, m = rr // M, rr % M
                    plo = b * Mq + max(0, m - (M - Mq))
                    phi = b * Mq + min(m, Mq - 1)
                    lo = plo if lo is None else min(lo, plo)
                    hi = phi if hi is None else max(hi, phi)
                return range(lo // 128, hi // 128 + 1)

            wins = [list(win(rc)) for rc in range(NT)]
            inv = [[rc for rc in range(NT) if cc in wins[rc]] for cc in range(NTq)]

            # ---- in-place gather of t (ascending cc; reads rc >= cc only)
            for cc in range(NTq):
                cq = rows_of(cc, Rq)
                rcs = inv[cc]
                stiles = []
                for rc in rcs:
                    cnt = rows_of(rc, R)
                    psh = sb.tile([128, 1], F32, tag="possh", name=f"ps{uid()}", bufs=4)
                    nc.vector.tensor_scalar(psh[:cnt, :],
                                            poscol[:cnt, 2 * rc:2 * rc + 1],
                                            -128.0 * cc, None, OP.add)
                    st = clsB(128, 128, F32R)
                    nc.vector.tensor_scalar(st[:cnt, :], iota_t[:cnt, :],
                                            psh[:cnt, :],
                                            poscol[:cnt, 2 * rc + 1:2 * rc + 2],
                                            OP.is_equal, OP.mult)
                    stiles.append((rc, cnt, st))
                gp = psB(128, D)
                for n0, nw in chunks(D, 512):
                    for ri, (rc, cnt, st) in enumerate(stiles):
                        nc.tensor.matmul(
                            gp[:, n0:n0 + nw], st[:cnt, :],
                            t_t[rc][:cnt, n0:n0 + nw],
                            start=(ri == 0), stop=(ri == len(stiles) - 1))
                nc.scalar.activation(t_t[cc][:cq, :], gp[:cq, :], AF.Copy)

            # ---- attention per pair on pruned tokens
            st2 = ln_stats(t_t, Rq, NTq)
            mkch = chunks(Mq, 128)
            Mq2 = Mq + (Mq % 2)  # even-padded free dim for fp32r matmuls
            for (w0, wl, b0) in pair_cols(Mq):
                xnp, xb = xn_cols(st2, w0, wl, Rq)
                xo = w0 - xb
                qp = [clsB(128, 640, F32R) for _ in range(6)]
                kp = [clsB(128, 640, F32R) for _ in range(6)]
                for dst, wh in ((qp, wqh), (kp, wkh)):
                    for nt in range(6):
                        pt = psA(128, wl)
                        for kt in range(6):
                            h2, k32 = kt // 3, kt % 3
                            nc.tensor.matmul(
                                pt[:, :wl],
                                wh[h2][:, D * k32 + 128 * nt:D * k32 + 128 * (nt + 1)],
                                xnp[kt][:, xo:xo + wl],
                                start=(kt == 0), stop=(kt == 5))
                        nc.scalar.activation(dst[nt][:, :wl], pt[:, :wl], AF.Copy)
                vt = {}
                for bi, b in enumerate((b0, b0 + 1)):
                    for ci, (m0, mw) in enumerate(mkch):
                        v1 = clsA(128, D, F32R)
                        pt = psB(128, D)
                        for n0, nw in chunks(D, 512):
                            for kt in range(6):
                                h2, k32 = kt // 3, kt % 3
                                nc.tensor.matmul(
                                    pt[:mw, n0:n0 + nw],
                                    xnp[kt][:, xo + bi * Mq + m0:xo + bi * Mq + m0 + mw],
                                    wvh[h2][:, D * k32 + n0:D * k32 + n0 + nw],
                                    start=(kt == 0), stop=(kt == 5))
                        nc.scalar.activation(v1[:mw, :], pt[:mw, :], AF.Copy)
                        vt[(b, ci)] = v1
                op6 = [clsB(128, wl, F32R) for _ in range(6)]
                for bi, b in enumerate((b0, b0 + 1)):
                    boff = bi * Mq
                    ptall = [sb.tile([128, H * Mq2], F32R, tag=f"pt{ci}",
                                     name=f"pt{uid()}",
                                     padded_shape=[128, H * (SCHED[0][1] + 1)])
                             for ci in range(len(mkch))]
                    for h in range(12):
                        kt, ro = h // 2, 64 * (h % 2)
                        for ci, (m0, mw) in enumerate(mkch):
                            sc = psA(128, wl)
                            nc.tensor.matmul(
                                sc[:, :wl],
                                kp[kt][ro:ro + 64, boff + m0:boff + m0 + 128],
                                qp[kt][ro:ro + 64, :wl],
                                start=True, stop=True)
                            nc.scalar.activation(
                                ptall[ci][:mw, h * Mq2:h * Mq2 + Mq],
                                sc[:mw, boff:boff + Mq], AF.Exp, scale=SC)
                            if Mq2 != Mq:
                                nc.vector.memset(
                                    ptall[ci][:mw, h * Mq2 + Mq:(h + 1) * Mq2].bitcast(F32),
                                    0.0)
                    invd = sb.tile([1, H * Mq2], F32R, tag="invd", name=f"iv{uid()}",
                                   padded_shape=[1, H * (SCHED[0][1] + 1)], bufs=1)
                    for d0, dw in chunks(H * Mq2, 512):
                        dn = psA(128, dw)
                        for ci, (m0, mw) in enumerate(mkch):
                            nc.tensor.matmul(dn[:, :], ones2dr_t[:mw, :],
                                             ptall[ci][:mw, d0:d0 + dw],
                                             start=(ci == 0),
                                             stop=(ci == len(mkch) - 1))
                        nc.vector.reciprocal(invd[:, d0:d0 + dw], dn[0:1, :])
                    for h in range(12):
                        kt, ro = h // 2, 64 * (h % 2)
                        op_ = psA(128, Mq2)
                        for ci, (m0, mw) in enumerate(mkch):
                            nc.tensor.matmul(op_[:, :],
                                             vt[(b, ci)][:mw, 128 * kt:128 * (kt + 1)],
                                             ptall[ci][:mw, h * Mq2:(h + 1) * Mq2],
                                             start=(ci == 0),
                                             stop=(ci == len(mkch) - 1))
                        ib = psA(128, Mq2)
                        nc.tensor.matmul(ib[:, :], onesr1_t[:, :],
                                         invd[:, h * Mq2:(h + 1) * Mq2],
                                         start=True, stop=True)
                        nc.scalar.activation(op6[kt][ro:ro + 64, boff:boff + Mq],
                                             op_[ro:ro + 64, :Mq], AF.Copy)
                        nc.vector.tensor_tensor(op6[kt][ro:ro + 64, boff:boff + Mq],
                                                op6[kt][ro:ro + 64, boff:boff + Mq],
                                                ib[0:64, :Mq], OP.mult)
                for kt in range(6):
                    nc.sync.dma_start(o_dram[128 * kt:128 * (kt + 1), w0:w0 + wl],
                                      op6[kt][:, :wl].bitcast(F32))

            # ---- WO (streamed from o_dram) + residual
            woh = [load_whalf(wo_d[l], f"o{l}", h) for h in range(2)]
            for cc in range(NTq):
                cq = rows_of(cc, Rq)
                oc = [clsB(128, 128, F32R) for _ in range(6)]
                for kt in range(6):
                    nc.sync.dma_start(
                        oc[kt][:, :cq],
                        o_dram[128 * kt:128 * (kt + 1), 128 * cc:128 * cc + cq]
                        .bitcast(F32R))
                wp_ = psB(128, D)
                for n0, nw in chunks(D, 512):
                    for kt in range(6):
                        h2, k32 = kt // 3, kt % 3
                        nc.tensor.matmul(
                            wp_[:, n0:n0 + nw],
                            oc[kt][:, :],
                            woh[h2][:, D * k32 + n0:D * k32 + n0 + nw],
                            start=(kt == 0), stop=(kt == 5))
                nc.vector.tensor_tensor(t_t[cc][:cq, :], wp_[:cq, :],
                                        t_t[cc][:cq, :], OP.add)

            # ---- LN2 -> xn2 on the weight-slot ring; then MLP
            st3 = ln_stats(t_t, Rq, NTq)
            xn2 = [sb.tile([128, Rq], F32R, tag="wslot", name=f"xn2_{l}_{kt}",
                           bufs=6, padded_shape=[128, 3 * D]) for kt in range(6)]
            for c0, cw in chunks(Rq, 512):
                sub, sb_ = xn_cols(st3, c0, cw, Rq)
                cwa = min(512, ((Rq + 127) // 128) * 128 - c0)
                cwv = min(cwa, Rq - c0)
                for kt in range(6):
                    nc.vector.tensor_copy(xn2[kt][:, c0:c0 + cwv], sub[kt][:, :cwv])
            GRP = 6
            for g0 in range(0, 24, GRP):
                gts = []
                for n1 in range(g0, g0 + GRP):
                    w1c = clsA(128, 6 * 128, F32R)
                    nc.sync.dma_start(
                        w1c[:].rearrange("p (k n) -> p k n", k=6),
                        w1_d[l, :, 128 * n1:128 * (n1 + 1)]
                        .rearrange("(k p) n -> p k n", p=128))
                    thirds = []
                    for c0, cw in chunks(Rq, 512):
                        gt = clsB(128, 512, F32R)
                        pt = psA(128, cw)
                        for kt in range(6):
                            nc.tensor.matmul(
                                pt[:, :cw], w1c[:, 128 * kt:128 * (kt + 1)],
                                xn2[kt][:, c0:c0 + cw],
                                start=(kt == 0), stop=(kt == 5))
                        nc.scalar.activation(gt[:, :cw], pt[:, :cw], AF.Gelu)
                        thirds.append((c0, gt))
                    gts.append(thirds)
                w2c = []
                for kt2 in range(g0, g0 + GRP):
                    wc = clsA(128, D, F32R)
                    nc.sync.dma_start(wc[:], w2_d[l, 128 * kt2:128 * (kt2 + 1), :])
                    w2c.append(wc)
                for cc in range(NTq):
                    cq = rows_of(cc, Rq)
                    wp2 = psB(128, D)
                    for n0, nw in chunks(D, 512):
                        for j in range(GRP):
                            c0, gsel = next(
                                (c0, g) for c0, g in gts[j]
                                if c0 <= 128 * cc < c0 + 512)
                            nc.tensor.matmul(
                                wp2[:, n0:n0 + nw],
                                gsel[:, 128 * cc - c0:128 * cc - c0 + 128],
                                w2c[j][:, n0:n0 + nw],
                                start=(j == 0), stop=(j == GRP - 1))
                    nc.vector.tensor_tensor(t_t[cc][:cq, :], wp2[:cq, :],
                                            t_t[cc][:cq, :], OP.add)

        # =========================================================
        # final LN + head on CLS rows
        # =========================================================
        Mf = SCHED[-1][1]
        Rf = B_LOC * Mf
        stf = ln_stats(t_t, Rf, cdiv(Rf, 128))
        rstdf = stf[:, NT0:NT0 + cdiv(Rf, 128)]
        nmursf = stf[:, 2 * NT0:2 * NT0 + cdiv(Rf, 128)]
        cls_raw = clsA(8, D, F32R)
        cls_st = sb.tile([8, 2], F32, tag="clsst", name="cls_st")
        for b in range(B_LOC):
            rr = b * Mf
            ti, off = rr // 128, rr % 128
            nc.sync.dma_start(cls_raw[b:b + 1, :], t_t[ti][off:off + 1, :])
            nc.sync.dma_start(cls_st[b:b + 1, 0:1], rstdf[off:off + 1, ti:ti + 1])
            nc.sync.dma_start(cls_st[b:b + 1, 1:2], nmursf[off:off + 1, ti:ti + 1])
        cls_tm = clsA(8, D, F32R)
        nc.scalar.activation(cls_tm[:8, :], cls_raw[:8, :], AF.Identity,
                             scale=cls_st[:8, 0:1], bias=cls_st[:8, 1:2])
        xcls_fm = [clsB(128, 128, F32R) for _ in range(6)]
        for kt in range(6):
            pt = psA(128, 8)
            nc.tensor.matmul(pt[:, 0:8].bitcast(F32R),
                             cls_tm[:8, 128 * kt:128 * (kt + 1)],
                             ident_t[:8, :8], is_transpose=True, start=True, stop=True)
            nc.vector.tensor_scalar(xcls_fm[kt][:, :], ones2dr_t[:].bitcast(F32), 0.0, None, OP.mult)
            nc.vector.tensor_copy(xcls_fm[kt][:, 0:8], pt[:, :].bitcast(F32R))
        out_sb = sb.tile([8, V_OUT], F32, tag="wslot", name="out_sb", bufs=6,
                         padded_shape=[128, 3 * D])
        for o0, ow in chunks(V_OUT, 512):
            hp = psA(128, ow)
            for kt in range(6):
                hwc = clsB(128, ow, F32R)
                nc.sync.dma_start(hwc[:, :ow], hw_d[128 * kt:128 * (kt + 1), o0:o0 + ow])
                nc.tensor.matmul(hp[:, :ow], xcls_fm[kt][:, :],
                                 hwc[:, :ow], start=(kt == 0), stop=(kt == 5))
            nc.scalar.activation(out_sb[:, o0:o0 + ow], hp[:8, :ow], AF.Copy)
        nc.sync.dma_start(out_d[:, :], out_sb[:])

    nc.compile()
    return nc


# =============================================================
# host side
# =============================================================
_CACHE = {}


def _consts():
    iota = np.broadcast_to(np.arange(128, dtype=np.float32), (128, 128)).copy()
    pcol = np.arange(128, dtype=np.float32).reshape(128, 1)
    ident = np.eye(128, dtype=np.float32)
    e1r = np.ones((128, 2), np.float32)
    e1r[0, :] = 0.0
    onesf = np.ones((128, 2), np.float32)
    onesr1 = np.ones((1, 128), np.float32)
    ones1 = np.ones((1, 128), np.float32)
    hmask = np.zeros((D, 128), np.float32)
    for d in range(D):
        hmask[d, (d % 128) // 64 + (d // 128) * 2] = 1.0
    ones2dr = np.ones((128, 128), np.float32)
    return dict(iota128f=iota, pcol128=pcol, ident128=ident,
                e1r128=e1r, onesf128=onesf, onesr_1x=onesr1, ones_1x=ones1,
                headmaskr=hmask, ones2dr=ones2dr)


def prepare_inputs(inputs):
    """Full inputs -> list of 8 per-core in_maps."""
    x = np.asarray(inputs["x"], np.float32)
    B = x.shape[0]
    M0 = SCHED[0][0]
    p = x.reshape(B, 3, GRID, PATCH, GRID, PATCH).transpose(0, 2, 4, 1, 3, 5)
    tokens = p.reshape(B, GRID * GRID, 3 * PATCH * PATCH)  # [B,196,768]
    pos = np.asarray(inputs["pos_emb"], np.float32).copy()  # [197,768]
    posf = pos.copy()
    posf[0] += np.asarray(inputs["cls_tok"], np.float32)
    posrep = np.tile(posf, (B_LOC, 1))  # [8*197, 768]

    w = {k: round_fp32r(np.asarray(inputs[k], np.float32))
         for k in ("patch_w", "wq", "wk", "wv", "wo", "w1", "w2", "head_w")}
    consts = _consts()
    consts["headmaskr"] = round_fp32r(consts["headmaskr"])

    in_maps = []
    for c in range(N_CORES):
        xt = np.zeros((B_LOC * M0, D), np.float32)
        for bl in range(B_LOC):
            xt[bl * M0 + 1:(bl + 1) * M0] = tokens[c * B_LOC + bl]
        m = dict(
            xtok_fm=np.ascontiguousarray(round_fp32r(xt).T),
            posrep=posrep,
            pw=w["patch_w"], wq=w["wq"], wk=w["wk"], wv=w["wv"], wo=w["wo"],
            w1=w["w1"], w2=w["w2"], hw=w["head_w"],
            **consts,
        )
        in_maps.append(m)
    return in_maps


def _get_runner():
    if "runner" not in _CACHE:
        import numpy as _np
        import jax
        from jax.sharding import Mesh, PartitionSpec, NamedSharding
        from jax.experimental.shard_map import shard_map
        import concourse.bass2jax as b2j
        import concourse.mybir as mybir

        nc = build_graph()
        b2j.install_neuronx_cc_hook()
        partition_name = nc.partition_id_tensor.name if nc.partition_id_tensor else None
        in_names, out_names, out_avals, zero_outs = [], [], [], []
        for alloc in nc.m.functions[0].allocations:
            if not isinstance(alloc, mybir.MemoryLocationSet):
                continue
            name = alloc.memorylocations[0].name
            if alloc.kind == "ExternalInput":
                if name != partition_name:
                    in_names.append(name)
            elif alloc.kind == "ExternalOutput":
                out_names.append(name)
                shape = tuple(alloc.tensor_shape)
                dtype = mybir.dt.np(alloc.dtype)
                out_avals.append(jax.core.ShapedArray(shape, dtype))
                zero_outs.append(_np.zeros(shape, dtype))
        n_params, n_outs = len(in_names), len(out_avals)
        all_in = list(in_names) + list(out_names)
        if partition_name:
            all_in.append(partition_name)

        def _body(*args):
            operands = list(args)
            if partition_name:
                operands.append(b2j.partition_id_tensor())
            return tuple(b2j._bass_exec_p.bind(
                *operands, out_avals=tuple(out_avals), in_names=tuple(all_in),
                out_names=tuple(out_names), lowering_input_output_aliases=(),
                sim_require_finite=True, sim_require_nnan=True, nc=nc))

        devices = jax.devices()[:N_CORES]
        mesh = Mesh(_np.asarray(devices), ("core",))
        sharded = jax.jit(
            shard_map(_body, mesh=mesh,
                      in_specs=(PartitionSpec("core"),) * (n_params + n_outs),
                      out_specs=(PartitionSpec("core"),) * n_outs,
                      check_rep=False),
            keep_unused=True)
        _CACHE["runner"] = (sharded, in_names, out_names, out_avals, zero_outs, mesh)
    return _CACHE["runner"]


def run_maps(in_maps):
    import jax
    import numpy as _np
    from jax.sharding import PartitionSpec, NamedSharding
    sharded, in_names, out_names, out_avals, zero_outs, mesh = _get_runner()
    per_core = [[_np.asarray(m[n]) for n in in_names] for m in in_maps]
    concat = [_np.concatenate([per_core[c][i] for c in range(N_CORES)], axis=0)
              for i in range(len(in_names))]
    concat += [_np.zeros((N_CORES * z.shape[0], *z.shape[1:]), z.dtype)
               for z in zero_outs]
    sh = NamedSharding(mesh, PartitionSpec("core"))
    handles = [jax.device_put(a, sh) for a in concat]
    outs = sharded(*handles)
    jax.block_until_ready(outs)
    return [
        {n: _np.asarray(outs[i]).reshape(N_CORES, *out_avals[i].shape)[c]
         for i, n in enumerate(out_names)}
        for c in range(N_CORES)
    ]


def kernel(**inputs):
    in_maps = prepare_inputs(inputs)
    res = run_maps(in_maps)
    return np.concatenate([res[c]["out"] for c in range(N_CORES)], axis=0)


# revision 29
# speedup vs baseline: 1.0206x; 1.0206x over previous
"""AdaptiveJacobianPrunedViT on 8 TRN2 NeuronCores (Bass/Tile, SPMD data-parallel).

Sharding: pure data parallel over batch (8 images/core). Token pruning uses
batch-mean importance; local importance vectors are AllReduce-summed across
cores before top-k so every core keeps the identical (reference) token set.

Per-core layouts:
  t (residual):  token-major flat [8*M, 768] as [128,768] f32r tiles
  xn/q/k/o:      feature-major [768, 8*M] as 6 [128, R] f32r tiles
  v:             token-major per-(b,chunk) [<=128, 768] tiles (PV lhsT)
All GEMMs run fp32r (11-bit RNE input rounding, fp32 accumulate).
Importance comparisons/broadcasts use exact fp32 matmuls.
"""
import numpy as np

MIN_TOKENS, KEEP_FRAC = 16, 0.85
H, HD, D, FF, L = 12, 64, 768, 3072, 12
B_LOC, N_CORES, LN_EPS, V_OUT = 8, 8, 1e-6, 1000
GRID, PATCH = 14, 16


def schedule():
    sched, M = [], GRID * GRID + 1
    for _ in range(L):
        Np = M - 1
        nxt = max(MIN_TOKENS, int(Np * KEEP_FRAC)) if Np > MIN_TOKENS else Np
        if Np > MIN_TOKENS and nxt < Np:
            sched.append((M, nxt + 1, nxt))
            M = nxt + 1
        else:
            sched.append((M, M, None))
    return sched


SCHED = schedule()


def round_fp32r(a):
    a = np.ascontiguousarray(a, dtype=np.float32)
    u = a.view(np.uint32).astype(np.uint64)
    sh = 12
    half, mask = np.uint64(1 << (sh - 1)), np.uint64((1 << sh) - 1)
    frac = u & mask
    u2 = u & ~mask
    up = (frac > half) | ((frac == half) & (((u >> np.uint64(sh)) & np.uint64(1)) == 1))
    u2 = u2 + np.where(up, np.uint64(1 << sh), np.uint64(0))
    return u2.astype(np.uint32).view(np.float32)


def cdiv(a, b):
    return (a + b - 1) // b


def chunks(total, step, start=0):
    return [(s, min(step, total - s)) for s in range(start, total, step)]


def build_graph():
    import concourse.bacc as bacc
    import concourse.mybir as mybir
    import concourse.tile as tile
    import contextlib

    F32, F32R = mybir.dt.float32, mybir.dt.float32r
    AF, OP = mybir.ActivationFunctionType, mybir.AluOpType

    M0 = SCHED[0][0]
    R0 = B_LOC * M0          # 1576
    NT0 = cdiv(R0, 128)      # 13
    SC = float(HD) ** -0.5

    nc = bacc.Bacc("TRN2", target_bir_lowering=False, debug=False, num_devices=N_CORES)

    xtok_d = nc.dram_tensor("xtok_fm", [D, R0], F32R, kind="ExternalInput")
    posrep_d = nc.dram_tensor("posrep", [R0, D], F32, kind="ExternalInput")
    pw_d = nc.dram_tensor("pw", [D, D], F32R, kind="ExternalInput")
    wq_d = nc.dram_tensor("wq", [L, D, D], F32R, kind="ExternalInput")
    wk_d = nc.dram_tensor("wk", [L, D, D], F32R, kind="ExternalInput")
    wv_d = nc.dram_tensor("wv", [L, D, D], F32R, kind="ExternalInput")
    wo_d = nc.dram_tensor("wo", [L, D, D], F32R, kind="ExternalInput")
    w1_d = nc.dram_tensor("w1", [L, D, FF], F32R, kind="ExternalInput")
    w2_d = nc.dram_tensor("w2", [L, FF, D], F32R, kind="ExternalInput")
    hw_d = nc.dram_tensor("hw", [D, V_OUT], F32R, kind="ExternalInput")
    iota_d = nc.dram_tensor("iota128f", [128, 128], F32, kind="ExternalInput")
    pcol_d = nc.dram_tensor("pcol128", [128, 1], F32, kind="ExternalInput")
    ident_d = nc.dram_tensor("ident128", [128, 128], F32R, kind="ExternalInput")
    e1r_d = nc.dram_tensor("e1r128", [128, 2], F32, kind="ExternalInput")
    onesf_d = nc.dram_tensor("onesf128", [128, 2], F32, kind="ExternalInput")
    onesr1_d = nc.dram_tensor("onesr_1x", [1, 128], F32R, kind="ExternalInput")
    ones1_d = nc.dram_tensor("ones_1x", [1, 128], F32, kind="ExternalInput")
    hmaskr_d = nc.dram_tensor("headmaskr", [D, 128], F32R, kind="ExternalInput")
    ones2dr_d = nc.dram_tensor("ones2dr", [128, 128], F32R, kind="ExternalInput")

    out_d = nc.dram_tensor("out", [B_LOC, V_OUT], F32, kind="ExternalOutput")

    with tile.TileContext(nc) as tc, contextlib.ExitStack() as ctx:
        ctx.enter_context(nc.allow_low_precision(reason="fp32r compute by design"))
        sb = ctx.enter_context(tc.tile_pool(name="sb", bufs=1))
        ps = ctx.enter_context(tc.tile_pool(name="ps", bufs=1, space="PSUM"))
        dram = ctx.enter_context(tc.tile_pool(name="dram", bufs=1, space="DRAM"))

        _uid = [0]

        def uid():
            _uid[0] += 1
            return _uid[0]

        def psA(p, f, name=None):
            return ps.tile([p, f], F32, tag="psA", name=f"pA{uid()}", bufs=4,
                           padded_shape=[128, 512])

        psQ = psA
        psI = psA

        def psB(p, f, name=None):
            return ps.tile([p, f], F32, tag="psB", name=f"pB{uid()}", bufs=2,
                           padded_shape=[128, 1024])

        # size-classed SBUF scratch: clsA = [128, <=864] f32r-ish slots
        def clsA(p, f, dtype, nbufs=8):
            return sb.tile([p, f], dtype, tag="clsA", name=f"cA{uid()}", bufs=nbufs,
                           padded_shape=[128, 768])

        def clsB(p, f, dtype, nbufs=18):
            return sb.tile([p, f], dtype, tag="clsB", name=f"cB{uid()}", bufs=nbufs,
                           padded_shape=[128, 640])

        # ---- consts
        iota_t = sb.tile([128, 128], F32, name="iota_t")
        pcol_t = sb.tile([128, 1], F32, name="pcol_t")
        nc.sync.dma_start(pcol_t[:], pcol_d[:, :])
        ident_t = sb.tile([128, 128], F32R, name="ident_t")
        e1r_t = sb.tile([128, 2], F32, name="e1r_t")
        onesf_t = sb.tile([128, 2], F32, name="onesf_t")
        onesr1_t = sb.tile([1, 128], F32R, name="onesr1_t")
        ones1_t = sb.tile([1, 128], F32, name="ones1_t")
        hmaskr_t = [sb.tile([128, 128], F32R, name=f"hmr{i}") for i in range(6)]
        ones2dr_t = sb.tile([128, 128], F32R, name="ones2dr_t")
        nc.sync.dma_start(ones2dr_t[:], ones2dr_d[:, :])
        nc.sync.dma_start(iota_t[:], iota_d[:, :])
        nc.sync.dma_start(ident_t[:], ident_d[:, :])
        nc.sync.dma_start(e1r_t[:], e1r_d[:, :])
        nc.sync.dma_start(onesf_t[:], onesf_d[:, :])
        nc.sync.dma_start(onesr1_t[:], onesr1_d[:, :])
        nc.sync.dma_start(ones1_t[:], ones1_d[:, :])
        for i in range(6):
            nc.sync.dma_start(hmaskr_t[i][:], hmaskr_d[128 * i:128 * (i + 1), :])
        eps_t = sb.tile([128, 1], F32, name="eps_t")
        nc.vector.memset(eps_t[:], LN_EPS)

        def rows_of(i, R):
            return min(128, R - 128 * i)

        # persistent residual tiles (13 x [128, 768] f32r)
        t_t = [sb.tile([128, D], F32R, tag=f"t{i}", name=f"t{i}") for i in range(NT0)]

        # weight slot: half-matrices [128, 3*768] (wq/wk/wv/wo halves), bufs=2
        def load_whalf(dram_ap, tag_l, half):
            t = sb.tile([128, 3 * D], F32R, tag="wslot", name=f"w{tag_l}_{half}_{uid()}",
                        bufs=6, padded_shape=[128, 3 * D])
            nc.sync.dma_start(
                t[:].rearrange("p (k n) -> p k n", k=3),
                dram_ap[128 * 3 * half:128 * 3 * (half + 1), :]
                .rearrange("(k p) n -> p k n", p=128))
            return t

        def ln_stats(tiles, R, nt_max):
            NT = cdiv(R, 128)
            sums = sb.tile([128, 2 * NT0], F32, tag="lnsums", name=f"sums{uid()}")
            junk = clsA(128, D, F32)
            for i in range(NT):
                r = rows_of(i, R)
                nc.scalar.activation(junk[:r, :], tiles[i][:r, :], AF.Identity,
                                     accum_out=sums[:r, i:i + 1])
                nc.scalar.activation(junk[:r, :], tiles[i][:r, :], AF.Square,
                                     accum_out=sums[:r, NT0 + i:NT0 + i + 1])
            stats = sb.tile([128, 4 * NT0], F32, tag="lnstats", name=f"stats{uid()}")
            mu = stats[:, 0:NT]
            rstd = stats[:, NT0:NT0 + NT]
            nmurs = stats[:, 2 * NT0:2 * NT0 + NT]
            scr = stats[:, 3 * NT0:3 * NT0 + NT]
            nc.vector.tensor_scalar(mu, sums[:, 0:NT], 1.0 / D, None, OP.mult)
            nc.vector.tensor_scalar(scr, sums[:, NT0:NT0 + NT], 1.0 / D, None, OP.mult)
            nc.vector.tensor_tensor(rstd, mu, mu, OP.mult)
            nc.vector.tensor_tensor(scr, scr, rstd, OP.subtract)      # var
            nc.scalar.activation(scr, scr, AF.Sqrt, bias=eps_t[:, :])  # std
            nc.vector.reciprocal(rstd, scr)
            nc.vector.tensor_tensor(nmurs, mu, rstd, OP.mult)
            nc.vector.tensor_scalar(nmurs, nmurs, -1.0, None, OP.mult)
            return stats

        def xn_cols(stats, c0, cw, R):
            """LN-applied xn feature-major covering cols [c0, c0+cw).
            Works on the 128-aligned covering range; returns (tiles, base)."""
            NT = cdiv(R, 128)
            rstd = stats[:, NT0:NT0 + NT]
            nmurs = stats[:, 2 * NT0:2 * NT0 + NT]
            t0a = (c0 // 128) * 128
            t1a = min(cdiv(c0 + cw, 128) * 128, ((R + 127) // 128) * 128)
            tis = list(range(t0a // 128, t1a // 128))
            cwa = sum(rows_of(ti, R) for ti in tis)
            dst = [clsB(128, cwa, F32R) for _ in range(6)]
            stage_list = []
            dl = 0
            for ti in tis:
                take = rows_of(ti, R)
                stage = clsA(128, D, F32R)
                nc.scalar.activation(stage[:take, :], t_t[ti][:take, :],
                                     AF.Identity,
                                     scale=rstd[:take, ti:ti + 1],
                                     bias=nmurs[:take, ti:ti + 1])
                stage_list.append((stage, take, dl))
                dl += take
            for kt in range(6):
                pt = psB(128, cwa)
                for j, (stage, take, dl) in enumerate(stage_list):
                    nc.tensor.matmul(
                        pt[:, dl:dl + take].bitcast(F32R),
                        stage[:take, 128 * kt:128 * (kt + 1)],
                        ident_t[:take, :take], is_transpose=True,
                        start=(j == 0), stop=(j == len(stage_list) - 1))
                nc.vector.tensor_copy(dst[kt][:, :cwa], pt[:, :cwa].bitcast(F32R))
            return dst, t0a

        def pair_cols(M):
            """[(w0, wlen, voffs per b in pair)] covering batches in pairs."""
            out = []
            for b0 in range(0, B_LOC, 2):
                out.append((b0 * M, 2 * M, b0))
            return out

        # =========================================================
        # patch embed: t = xtok @ pw + posrep
        # =========================================================
        pwh = [load_whalf(pw_d[:, :], "pw", h) for h in range(2)]
        for i in range(NT0):
            r = rows_of(i, R0)
            xtk = [clsB(128, 128, F32R) for _ in range(6)]
            for kt in range(6):
                nc.sync.dma_start(xtk[kt][:, :r],
                                  xtok_d[128 * kt:128 * (kt + 1), 128 * i:128 * i + r])
            pos_t = clsA(128, D, F32)
            nc.sync.dma_start(pos_t[:r, :], posrep_d[128 * i:128 * i + r, :])
            pt = psB(128, D)
            for n0, nw in chunks(D, 512):
                for kt in range(6):
                    half, k3 = kt // 3, kt % 3
                    nc.tensor.matmul(
                        pt[:, n0:n0 + nw],
                        xtk[kt][:, :],
                        pwh[half][:, D * k3 + n0:D * k3 + n0 + nw],
                        start=(kt == 0), stop=(kt == 5))
            nc.vector.tensor_tensor(t_t[i][:r, :], pt[:r, :], pos_t[:r, :], OP.add)

        # =========================================================
        # transformer layers
        # =========================================================
        o_dram = dram.tile([D, R0], F32, tag="odram", name="o_dram")
        for l in range(L):
            M, Mq, n_next = SCHED[l]
            R, Rq = B_LOC * M, B_LOC * Mq
            NT, NTq = cdiv(R, 128), cdiv(Rq, 128)

            st1 = ln_stats(t_t, R, NT)
            wkh = [load_whalf(wk_d[l], f"k{l}", h) for h in range(2)]
            wvh = [load_whalf(wv_d[l], f"v{l}", h) for h in range(2)]
            wqh = [load_whalf(wq_d[l], f"q{l}", h) for h in range(2)]

            den = sb.tile([12, 16], F32, tag="den", name=f"den{l}")
            qc_sb = sb.tile([128, 12], F32, tag="qc", name=f"qc{l}", bufs=1)
            qcbd = sb.tile([128, 128], F32R, tag="qcbd", name=f"qcbd{l}", bufs=2)
            imp_ps = psI(1, M)
            for pi, (w0, wl, b0) in enumerate(pair_cols(M)):
                xnp, xb = xn_cols(st1, w0, wl, R)
                xo = w0 - xb
                qc_ps = psQ(128, 12)
                for nt in range(6):
                    for kt in range(6):
                        half, k3 = kt // 3, kt % 3
                        nc.tensor.matmul(
                            qc_ps[:, 2 * nt:2 * nt + 2],
                            wqh[half][:, D * k3 + 128 * nt:D * k3 + 128 * (nt + 1)],
                            xnp[kt][:, xo:xo + wl:M],
                            start=(nt == 0 and kt == 0), stop=(nt == 5 and kt == 5))
                nc.scalar.activation(qc_sb[:], qc_ps[:], AF.Copy)
                kpre = [clsB(128, wl, F32R) for _ in range(6)]
                vpre = [clsB(128, wl, F32R) for _ in range(6)]
                for dst, wh in ((kpre, wkh), (vpre, wvh)):
                    for nt in range(6):
                        pt = psA(128, wl)
                        for kt in range(6):
                            h2, k32 = kt // 3, kt % 3
                            nc.tensor.matmul(
                                pt[:, :wl],
                                wh[h2][:, D * k32 + 128 * nt:D * k32 + 128 * (nt + 1)],
                                xnp[kt][:, xo:xo + wl],
                                start=(kt == 0), stop=(kt == 5))
                        nc.scalar.activation(dst[nt][:, :wl], pt[:, :wl], AF.Copy)
                # vnorm for the pair
                vp = psA(128, wl)
                sqv = clsB(128, wl, F32R)
                for kt in range(6):
                    nc.scalar.activation(sqv[:, :wl], vpre[kt][:, :wl], AF.Square)
                    nc.tensor.matmul(vp[:, :wl], hmaskr_t[kt][:], sqv[:, :wl],
                                     start=(kt == 0), stop=(kt == 5))
                vnp = clsB(12, wl, F32)
                nc.scalar.activation(vnp[:12, :wl], vp[:12, :wl], AF.Sqrt)
                # CLS attention -> importance contribution
                pclsp = clsB(12, wl, F32)
                for bi, b in enumerate((b0, b0 + 1)):
                    sc = psA(128, wl)
                    for kt in range(6):
                        nc.vector.tensor_scalar(qcbd[:], hmaskr_t[kt][:].bitcast(F32),
                                                qc_sb[:, 2 * kt + bi:2 * kt + bi + 1],
                                                None, OP.mult)
                        nc.tensor.matmul(sc[:, :wl], qcbd[:], kpre[kt][:, :wl],
                                         start=(kt == 0), stop=(kt == 5))
                    voff = bi * M
                    nc.scalar.activation(pclsp[:12, voff:voff + M],
                                         sc[:12, voff:voff + M], AF.Exp, scale=SC)
                    nc.vector.tensor_reduce(den[:, b:b + 1],
                                            pclsp[:12, voff:voff + M],
                                            mybir.AxisListType.X, OP.add)
                nc.vector.reciprocal(den[:, 8 + b0:10 + b0], den[:, b0:b0 + 2])
                for bi, b in enumerate((b0, b0 + 1)):
                    voff = bi * M
                    nc.vector.tensor_scalar(vnp[:12, voff:voff + M],
                                            vnp[:12, voff:voff + M],
                                            den[:, 8 + b:9 + b], None, OP.mult)
                nc.vector.tensor_tensor(pclsp[:12, :wl], pclsp[:12, :wl],
                                        vnp[:12, :wl], OP.mult)
                for bi, b in enumerate((b0, b0 + 1)):
                    nc.tensor.matmul(imp_ps[:, :], onesf_t[:12, 0:1],
                                     pclsp[:12, bi * M:bi * M + M],
                                     start=(b == 0), stop=(b == B_LOC - 1))
            # ---- AllReduce importance
            imp_sb = sb.tile([1, 200], F32, tag="imp", name=f"imp{l}")
            nc.vector.tensor_copy(imp_sb[:, :M], imp_ps[:, :])
            bin_t = dram.tile([1, M], F32, tag="arin", name=f"arin{l}")
            bout_t = dram.tile([1, M], F32, tag="arout", name=f"arout{l}",
                               addr_space="Shared")
            nc.sync.dma_start(bin_t[:], imp_sb[:, :M])
            nc.gpsimd.collective_compute(
                "AllReduce", OP.add, replica_groups=[list(range(N_CORES))],
                ins=[bin_t.opt()], outs=[bout_t.opt()])
            impg = sb.tile([1, 200], F32, tag="impg", name=f"impg{l}")
            nc.sync.dma_start(impg[:, :M], bout_t[:])

            # ---- ranks -> mask -> pos
            KC = cdiv(M, 128)
            impcol = sb.tile([128, 2], F32, tag="impcol", name=f"impcol{l}")
            for kc in range(KC):
                cnt = rows_of(kc, M)
                icp = psA(128, 1)
                nc.tensor.matmul(icp[:cnt, :], impg[:, 128 * kc:128 * kc + cnt],
                                 ones1_t[:, 0:1], start=True, stop=True)
                nc.vector.tensor_copy(impcol[:cnt, kc:kc + 1], icp[:cnt, :])
            rank_ps = psI(1, M)
            for kc in range(KC):
                cnt = rows_of(kc, M)
                ibc = psA(128, M)
                nc.tensor.matmul(ibc[:cnt, :], ones1_t[:, :cnt], impg[:, :M],
                                 start=True, stop=True)
                Ct = clsB(128, M, F32)
                eqt = clsB(128, M, F32)
                C2t = clsB(128, M, F32)
                nc.vector.tensor_scalar(Ct[:cnt, :M], ibc[:cnt, :M],
                                        impcol[:cnt, kc:kc + 1], None, OP.is_lt)
                nc.vector.tensor_scalar(eqt[:cnt, :M], ibc[:cnt, :M],
                                        impcol[:cnt, kc:kc + 1], None, OP.is_equal)
                # C2t mask: (global col j) > (global row k = 128*kc + p)
                pshk = sb.tile([128, 1], F32, tag="possh", name=f"pk{uid()}", bufs=4)
                for j0, jw in chunks(M, 128):
                    nc.vector.tensor_scalar(pshk[:cnt, :], pcol_t[:cnt, :],
                                            float(128 * kc - j0), None, OP.add)
                    nc.vector.tensor_scalar(C2t[:cnt, j0:j0 + jw],
                                            iota_t[:cnt, :jw], pshk[:cnt, :],
                                            None, OP.is_gt)
                nc.vector.tensor_tensor(C2t[:cnt, :M], eqt[:cnt, :M],
                                        C2t[:cnt, :M], OP.mult)
                lhs = e1r_t if kc == 0 else onesf_t
                nc.tensor.matmul(rank_ps[:, :], lhs[:cnt, 0:1], Ct[:cnt, :M],
                                 start=(kc == 0), stop=False)
                nc.tensor.matmul(rank_ps[:, :], lhs[:cnt, 0:1], C2t[:cnt, :M],
                                 start=False, stop=(kc == KC - 1))
            mask = sb.tile([1, 200], F32R, tag="mask", name=f"mask{l}")
            nc.vector.tensor_scalar(mask[:, :M], rank_ps[:, :], float(n_next), None,
                                    OP.is_lt)
            nc.vector.tensor_copy(mask[:, 0:1], onesf_t[0:1, 0:1])
            mflat = sb.tile([1, R0], F32R, tag="mflat", name=f"mflat{l}")
            for b in range(B_LOC):
                nc.vector.tensor_copy(mflat[:, b * M:(b + 1) * M], mask[:, :M])
            cum = sb.tile([1, R0], F32, tag="cum", name=f"cum{l}")
            nc.vector.tensor_tensor_scan(cum[:, :R], mflat[:, :R].bitcast(F32),
                                         mflat[:, :R].bitcast(F32), 0.0,
                                         OP.add, OP.max)
            nc.vector.tensor_tensor(cum[:, :R], cum[:, :R], mflat[:, :R].bitcast(F32),
                                    OP.subtract)  # exclusive positions, in place
            poscol = sb.tile([128, 2 * NT0], F32, tag="poscol", name=f"poscol{l}")
            for rc in range(NT):
                cnt = rows_of(rc, R)
                pcp = psA(128, 2)
                nc.tensor.matmul(pcp[:cnt, 0:1], cum[:, 128 * rc:128 * rc + cnt],
                                 ones1_t[:, 0:1], start=True, stop=False)
                nc.tensor.matmul(pcp[:cnt, 1:2],
                                 mflat[:, 128 * rc:128 * rc + cnt].bitcast(F32),
                                 ones1_t[:, 0:1], start=False, stop=True)
                nc.vector.tensor_copy(poscol[:cnt, 2 * rc:2 * rc + 2], pcp[:cnt, :])

            def win(rc):
                lo, hi = None, None
                for rr in range(128 * rc, min(128 * rc + 128, R)):
                    b, m = rr // M, rr % M
                    plo = b * Mq + max(0, m - (M - Mq))
                    phi = b * Mq + min(m, Mq - 1)
                    lo = plo if lo is None else min(lo, plo)
                    hi = phi if hi is None else max(hi, phi)
                return range(lo // 128, hi // 128 + 1)

            wins = [list(win(rc)) for rc in range(NT)]
            inv = [[rc for rc in range(NT) if cc in wins[rc]] for cc in range(NTq)]

            # ---- in-place gather of t (ascending cc; reads rc >= cc only)
            for cc in range(NTq):
                cq = rows_of(cc, Rq)
                rcs = inv[cc]
                stiles = []
                for rc in rcs:
                    cnt = rows_of(rc, R)
                    psh = sb.tile([128, 1], F32, tag="possh", name=f"ps{uid()}", bufs=4)
                    nc.vector.tensor_scalar(psh[:cnt, :],
                                            poscol[:cnt, 2 * rc:2 * rc + 1],
                                            -128.0 * cc, None, OP.add)
                    st = clsB(128, 128, F32R)
                    nc.vector.tensor_scalar(st[:cnt, :], iota_t[:cnt, :],
                                            psh[:cnt, :],
                                            poscol[:cnt, 2 * rc + 1:2 * rc + 2],
                                            OP.is_equal, OP.mult)
                    stiles.append((rc, cnt, st))
                gp = psB(128, D)
                for n0, nw in chunks(D, 512):
                    for ri, (rc, cnt, st) in enumerate(stiles):
                        nc.tensor.matmul(
                            gp[:, n0:n0 + nw], st[:cnt, :],
                            t_t[rc][:cnt, n0:n0 + nw],
                            start=(ri == 0), stop=(ri == len(stiles) - 1))
                nc.scalar.activation(t_t[cc][:cq, :], gp[:cq, :], AF.Copy)

            # ---- attention per pair on pruned tokens
            st2 = ln_stats(t_t, Rq, NTq)
            mkch = chunks(Mq, 128)
            Mq2 = Mq + (Mq % 2)  # even-padded free dim for fp32r matmuls
            for (w0, wl, b0) in pair_cols(Mq):
                xnp, xb = xn_cols(st2, w0, wl, Rq)
                xo = w0 - xb
                qp = [clsB(128, 640, F32R) for _ in range(6)]
                kp = [clsB(128, 640, F32R) for _ in range(6)]
                for dst, wh in ((qp, wqh), (kp, wkh)):
                    for nt in range(6):
                        pt = psA(128, wl)
                        for kt in range(6):
                            h2, k32 = kt // 3, kt % 3
                            nc.tensor.matmul(
                                pt[:, :wl],
                                wh[h2][:, D * k32 + 128 * nt:D * k32 + 128 * (nt + 1)],
                                xnp[kt][:, xo:xo + wl],
                                start=(kt == 0), stop=(kt == 5))
                        nc.scalar.activation(dst[nt][:, :wl], pt[:, :wl], AF.Copy)
                vt = {}
                for bi, b in enumerate((b0, b0 + 1)):
                    for ci, (m0, mw) in enumerate(mkch):
                        v1 = clsA(128, D, F32R)
                        pt = psB(128, D)
                        for n0, nw in chunks(D, 512):
                            for kt in range(6):
                                h2, k32 = kt // 3, kt % 3
                                nc.tensor.matmul(
                                    pt[:mw, n0:n0 + nw],
                                    xnp[kt][:, xo + bi * Mq + m0:xo + bi * Mq + m0 + mw],
                                    wvh[h2][:, D * k32 + n0:D * k32 + n0 + nw],
                                    start=(kt == 0), stop=(kt == 5))
                        nc.scalar.activation(v1[:mw, :], pt[:mw, :], AF.Copy)
                        vt[(b, ci)] = v1
                op6 = [clsB(128, wl, F32R) for _ in range(6)]
                for bi, b in enumerate((b0, b0 + 1)):
                    boff = bi * Mq
                    ptall = [sb.tile([128, H * Mq2], F32R, tag=f"pt{ci}",
                                     name=f"pt{uid()}",
                                     padded_shape=[128, H * (SCHED[0][1] + 1)])
                             for ci in range(len(mkch))]
                    for h in range(12):
                        kt, ro = h // 2, 64 * (h % 2)
                        for ci, (m0, mw) in enumerate(mkch):
                            sc = psA(128, wl)
                            nc.tensor.matmul(
                                sc[:, :wl],
                                kp[kt][ro:ro + 64, boff + m0:boff + m0 + 128],
                                qp[kt][ro:ro + 64, :wl],
                                start=True, stop=True)
                            nc.scalar.activation(
                                ptall[ci][:mw, h * Mq2:h * Mq2 + Mq],
                                sc[:mw, boff:boff + Mq], AF.Exp, scale=SC)
                            if Mq2 != Mq:
                                nc.vector.memset(
                                    ptall[ci][:mw, h * Mq2 + Mq:(h + 1) * Mq2].bitcast(F32),
                                    0.0)
                    invd = sb.tile([1, H * Mq2], F32R, tag="invd", name=f"iv{uid()}",
                                   padded_shape=[1, H * (SCHED[0][1] + 1)], bufs=1)
                    for d0, dw in chunks(H * Mq2, 512):
                        dn = psA(128, dw)
                        for ci, (m0, mw) in enumerate(mkch):
                            nc.tensor.matmul(dn[:, :], ones2dr_t[:mw, :],
                                             ptall[ci][:mw, d0:d0 + dw],
                                             start=(ci == 0),
                                             stop=(ci == len(mkch) - 1))
                        nc.vector.reciprocal(invd[:, d0:d0 + dw], dn[0:1, :])
                    for h in range(12):
                        kt, ro = h // 2, 64 * (h % 2)
                        op_ = psA(128, Mq2)
                        for ci, (m0, mw) in enumerate(mkch):
                            nc.tensor.matmul(op_[:, :],
                                             vt[(b, ci)][:mw, 128 * kt:128 * (kt + 1)],
                                             ptall[ci][:mw, h * Mq2:(h + 1) * Mq2],
                                             start=(ci == 0),
                                             stop=(ci == len(mkch) - 1))
                        ib = psA(128, Mq2)
                        nc.tensor.matmul(ib[:, :], onesr1_t[:, :],
                                         invd[:, h * Mq2:(h + 1) * Mq2],
                                         start=True, stop=True)
                        nc.scalar.activation(op6[kt][ro:ro + 64, boff:boff + Mq],
                                             op_[ro:ro + 64, :Mq], AF.Copy)
                        nc.vector.tensor_tensor(op6[kt][ro:ro + 64, boff:boff + Mq],
                                                op6[kt][ro:ro + 64, boff:boff + Mq],
                                                ib[0:64, :Mq], OP.mult)
                for kt in range(6):
                    nc.sync.dma_start(o_dram[128 * kt:128 * (kt + 1), w0:w0 + wl],
                                      op6[kt][:, :wl].bitcast(F32))

            # ---- WO (streamed from o_dram) + residual
            woh = [load_whalf(wo_d[l], f"o{l}", h) for h in range(2)]
            for cc in range(NTq):
                cq = rows_of(cc, Rq)
                oc = [clsB(128, 128, F32R) for _ in range(6)]
                for kt in range(6):
                    nc.sync.dma_start(
                        oc[kt][:, :cq],
                        o_dram[128 * kt:128 * (kt + 1), 128 * cc:128 * cc + cq]
                        .bitcast(F32R))
                wp_ = psB(128, D)
                for n0, nw in chunks(D, 512):
                    for kt in range(6):
                        h2, k32 = kt // 3, kt % 3
                        nc.tensor.matmul(
                            wp_[:, n0:n0 + nw],
                            oc[kt][:, :],
                            woh[h2][:, D * k32 + n0:D * k32 + n0 + nw],
                            start=(kt == 0), stop=(kt == 5))
                nc.vector.tensor_tensor(t_t[cc][:cq, :], wp_[:cq, :],
                                        t_t[cc][:cq, :], OP.add)

            # ---- LN2 -> xn2 on the weight-slot ring; then MLP
            st3 = ln_stats(t_t, Rq, NTq)
            xn2 = [sb.tile([128, Rq], F32R, tag="wslot", name=f"xn2_{l}_{kt}",
                           bufs=6, padded_shape=[128, 3 * D]) for kt in range(6)]
            for c0, cw in chunks(Rq, 512):
                sub, sb_ = xn_cols(st3, c0, cw, Rq)
                cwa = min(512, ((Rq + 127) // 128) * 128 - c0)
                cwv = min(cwa, Rq - c0)
                for kt in range(6):
                    nc.vector.tensor_copy(xn2[kt][:, c0:c0 + cwv], sub[kt][:, :cwv])
            GRP = 6
            for g0 in range(0, 24, GRP):
                gts = []
                for n1 in range(g0, g0 + GRP):
                    w1c = clsA(128, 6 * 128, F32R)
                    nc.sync.dma_start(
                        w1c[:].rearrange("p (k n) -> p k n", k=6),
                        w1_d[l, :, 128 * n1:128 * (n1 + 1)]
                        .rearrange("(k p) n -> p k n", p=128))
                    thirds = []
                    for c0, cw in chunks(Rq, 512):
                        gt = clsB(128, 512, F32R)
                        pt = psA(128, cw)
                        for kt in range(6):
                            nc.tensor.matmul(
                                pt[:, :cw], w1c[:, 128 * kt:128 * (kt + 1)],
                                xn2[kt][:, c0:c0 + cw],
                                start=(kt == 0), stop=(kt == 5))
                        nc.scalar.activation(gt[:, :cw], pt[:, :cw], AF.Gelu)
                        thirds.append((c0, gt))
                    gts.append(thirds)
                w2c = []
                for kt2 in range(g0, g0 + GRP):
                    wc = clsA(128, D, F32R)
                    nc.sync.dma_start(wc[:], w2_d[l, 128 * kt2:128 * (kt2 + 1), :])
                    w2c.append(wc)
                for cc in range(NTq):
                    cq = rows_of(cc, Rq)
                    wp2 = psB(128, D)
                    for n0, nw in chunks(D, 512):
                        for j in range(GRP):
                            c0, gsel = next(
                                (c0, g) for c0, g in gts[j]
                                if c0 <= 128 * cc < c0 + 512)
                            nc.tensor.matmul(
                                wp2[:, n0:n0 + nw],
                                gsel[:, 128 * cc - c0:128 * cc - c0 + 128],
                                w2c[j][:, n0:n0 + nw],
                                start=(j == 0), stop=(j == GRP - 1))
                    nc.vector.tensor_tensor(t_t[cc][:cq, :], wp2[:cq, :],
                                            t_t[cc][:cq, :], OP.add)

        # =========================================================
        # final LN + head on CLS rows
        # =========================================================
        Mf = SCHED[-1][1]
        Rf = B_LOC * Mf
        stf = ln_stats(t_t, Rf, cdiv(Rf, 128))
        rstdf = stf[:, NT0:NT0 + cdiv(Rf, 128)]
        nmursf = stf[:, 2 * NT0:2 * NT0 + cdiv(Rf, 128)]
        cls_raw = clsA(8, D, F32R)
        cls_st = sb.tile([8, 2], F32, tag="clsst", name="cls_st")
        for b in range(B_LOC):
            rr = b * Mf
            ti, off = rr // 128, rr % 128
            nc.sync.dma_start(cls_raw[b:b + 1, :], t_t[ti][off:off + 1, :])
            nc.sync.dma_start(cls_st[b:b + 1, 0:1], rstdf[off:off + 1, ti:ti + 1])
            nc.sync.dma_start(cls_st[b:b + 1, 1:2], nmursf[off:off + 1, ti:ti + 1])
        cls_tm = clsA(8, D, F32R)
        nc.scalar.activation(cls_tm[:8, :], cls_raw[:8, :], AF.Identity,
                             scale=cls_st[:8, 0:1], bias=cls_st[:8, 1:2])
        xcls_fm = [clsB(128, 128, F32R) for _ in range(6)]
        for kt in range(6):
            pt = psA(128, 8)
            nc.tensor.matmul(pt[:, 0:8].bitcast(F32R),
                             cls_tm[:8, 128 * kt:128 * (kt + 1)],
                             ident_t[:8, :8], is_transpose=True, start=True, stop=True)
            nc.vector.tensor_scalar(xcls_fm[kt][:, :], ones2dr_t[:].bitcast(F32), 0.0, None, OP.mult)
            nc.vector.tensor_copy(xcls_fm[kt][:, 0:8], pt[:, :].bitcast(F32R))
        out_sb = sb.tile([8, V_OUT], F32, tag="wslot", name="out_sb", bufs=6,
                         padded_shape=[128, 3 * D])
        for o0, ow in chunks(V_OUT, 512):
            hp = psA(128, ow)
            for kt in range(6):
                hwc = clsB(128, ow, F32R)
                nc.sync.dma_start(hwc[:, :ow], hw_d[128 * kt:128 * (kt + 1), o0:o0 + ow])
                nc.tensor.matmul(hp[:, :ow], xcls_fm[kt][:, :],
                                 hwc[:, :ow], start=(kt == 0), stop=(kt == 5))
            nc.scalar.activation(out_sb[:, o0:o0 + ow], hp[:8, :ow], AF.Copy)
        nc.sync.dma_start(out_d[:, :], out_sb[:])

    nc.compile()
    return nc


# =============================================================
# host side
# =============================================================
_CACHE = {}


def _consts():
    iota = np.broadcast_to(np.arange(128, dtype=np.float32), (128, 128)).copy()
    pcol = np.arange(128, dtype=np.float32).reshape(128, 1)
    ident = np.eye(128, dtype=np.float32)
    e1r = np.ones((128, 2), np.float32)
    e1r[0, :] = 0.0
    onesf = np.ones((128, 2), np.float32)
    onesr1 = np.ones((1, 128), np.float32)
    ones1 = np.ones((1, 128), np.float32)
    hmask = np.zeros((D, 128), np.float32)
    for d in range(D):
        hmask[d, (d % 128) // 64 + (d // 128) * 2] = 1.0
    ones2dr = np.ones((128, 128), np.float32)
    return dict(iota128f=iota, pcol128=pcol, ident128=ident,
                e1r128=e1r, onesf128=onesf, onesr_1x=onesr1, ones_1x=ones1,
                headmaskr=hmask, ones2dr=ones2dr)


def prepare_inputs(inputs):
    """Full inputs -> list of 8 per-core in_maps."""
    x = np.asarray(inputs["x"], np.float32)
    B = x.shape[0]
    M0 = SCHED[0][0]
    p = x.reshape(B, 3, GRID, PATCH, GRID, PATCH).transpose(0, 2, 4, 1, 3, 5)
    tokens = p.reshape(B, GRID * GRID, 3 * PATCH * PATCH)  # [B,196,768]
    pos = np.asarray(inputs["pos_emb"], np.float32).copy()  # [197,768]
    posf = pos.copy()
    posf[0] += np.asarray(inputs["cls_tok"], np.float32)
    posrep = np.tile(posf, (B_LOC, 1))  # [8*197, 768]

    w = {k: round_fp32r(np.asarray(inputs[k], np.float32))
         for k in ("patch_w", "wq", "wk", "wv", "wo", "w1", "w2", "head_w")}
    consts = _consts()
    consts["headmaskr"] = round_fp32r(consts["headmaskr"])

    in_maps = []
    for c in range(N_CORES):
        xt = np.zeros((B_LOC * M0, D), np.float32)
        for bl in range(B_LOC):
            xt[bl * M0 + 1:(bl + 1) * M0] = tokens[c * B_LOC + bl]
        m = dict(
            xtok_fm=np.ascontiguousarray(round_fp32r(xt).T),
            posrep=posrep,
            pw=w["patch_w"], wq=w["wq"], wk=w["wk"], wv=w["wv"], wo=w["wo"],
            w1=w["w1"], w2=w["w2"], hw=w["head_w"],
            **consts,
        )
        in_maps.append(m)
    return in_maps


def _get_runner():
    if "runner" not in _CACHE:
        import numpy as _np
        import jax
        from jax.sharding import Mesh, PartitionSpec, NamedSharding
        from jax.experimental.shard_map import shard_map
        import concourse.bass2jax as b2j
        import concourse.mybir as mybir

        nc = build_graph()
        b2j.install_neuronx_cc_hook()
        partition_name = nc.partition_id_tensor.name if nc.partition_id_tensor else None
        in_names, out_names, out_avals, zero_outs = [], [], [], []
        for alloc in nc.m.functions[0].allocations:
            if not isinstance(alloc, mybir.MemoryLocationSet):
                continue
            name = alloc.memorylocations[0].name
            if alloc.kind == "ExternalInput":
                if name != partition_name:
                    in_names.append(name)
            elif alloc.kind == "ExternalOutput":
                out_names.append(name)
                shape = tuple(alloc.tensor_shape)
                dtype = mybir.dt.np(alloc.dtype)
                out_avals.append(jax.core.ShapedArray(shape, dtype))
                zero_outs.append(_np.zeros(shape, dtype))
        n_params, n_outs = len(in_names), len(out_avals)
        all_in = list(in_names) + list(out_names)
        if partition_name:
            all_in.append(partition_name)

        def _body(*args):
            operands = list(args)
            if partition_name:
                operands.append(b2j.partition_id_tensor())
            return tuple(b2j._bass_exec_p.bind(
                *operands, out_avals=tuple(out_avals), in_names=tuple(all_in),
                out_names=tuple(out_names), lowering_input_output_aliases=(),
                sim_require_finite=True, sim_require_nnan=True, nc=nc))

        devices = jax.devices()[:N_CORES]
        mesh = Mesh(_np.asarray(devices), ("core",))
        sharded = jax.jit(
            shard_map(_body, mesh=mesh,
                      in_specs=(PartitionSpec("core"),) * (n_params + n_outs),
                      out_specs=(PartitionSpec("core"),) * n_outs,
                      check_rep=False),
            keep_unused=True)
        _CACHE["runner"] = (sharded, in_names, out_names, out_avals, zero_outs, mesh)
    return _CACHE["runner"]


def run_maps(in_maps):
    import jax
    import numpy as _np
    from jax.sharding import PartitionSpec, NamedSharding
    sharded, in_names, out_names, out_avals, zero_outs, mesh = _get_runner()
    per_core = [[_np.asarray(m[n]) for n in in_names] for m in in_maps]
    concat = [_np.concatenate([per_core[c][i] for c in range(N_CORES)], axis=0)
              for i in range(len(in_names))]
    concat += [_np.zeros((N_CORES * z.shape[0], *z.shape[1:]), z.dtype)
               for z in zero_outs]
    sh = NamedSharding(mesh, PartitionSpec("core"))
    handles = [jax.device_put(a, sh) for a in concat]
    outs = sharded(*handles)
    jax.block_until_ready(outs)
    return [
        {n: _np.asarray(outs[i]).reshape(N_CORES, *out_avals[i].shape)[c]
         for i, n in enumerate(out_names)}
        for c in range(N_CORES)
    ]


def kernel(**inputs):
    in_maps = prepare_inputs(inputs)
    res = run_maps(in_maps)
    return np.concatenate([res[c]["out"] for c in range(N_CORES)], axis=0)


# revision 33
# speedup vs baseline: 1.0474x; 1.0263x over previous
"""AdaptiveJacobianPrunedViT on 8 TRN2 NeuronCores (Bass/Tile, SPMD data-parallel).

Sharding: pure data parallel over batch (8 images/core). Token pruning uses
batch-mean importance; local importance vectors are AllReduce-summed across
cores before top-k so every core keeps the identical (reference) token set.

Per-core layouts:
  t (residual):  token-major flat [8*M, 768] as [128,768] f32r tiles
  xn/q/k/o:      feature-major [768, 8*M] as 6 [128, R] f32r tiles
  v:             token-major per-(b,chunk) [<=128, 768] tiles (PV lhsT)
All GEMMs run fp32r (11-bit RNE input rounding, fp32 accumulate).
Importance comparisons/broadcasts use exact fp32 matmuls.
"""
import numpy as np

MIN_TOKENS, KEEP_FRAC = 16, 0.85
H, HD, D, FF, L = 12, 64, 768, 3072, 12
B_LOC, N_CORES, LN_EPS, V_OUT = 8, 8, 1e-6, 1000
GRID, PATCH = 14, 16


def schedule():
    sched, M = [], GRID * GRID + 1
    for _ in range(L):
        Np = M - 1
        nxt = max(MIN_TOKENS, int(Np * KEEP_FRAC)) if Np > MIN_TOKENS else Np
        if Np > MIN_TOKENS and nxt < Np:
            sched.append((M, nxt + 1, nxt))
            M = nxt + 1
        else:
            sched.append((M, M, None))
    return sched


SCHED = schedule()


def round_fp32r(a):
    a = np.ascontiguousarray(a, dtype=np.float32)
    u = a.view(np.uint32).astype(np.uint64)
    sh = 12
    half, mask = np.uint64(1 << (sh - 1)), np.uint64((1 << sh) - 1)
    frac = u & mask
    u2 = u & ~mask
    up = (frac > half) | ((frac == half) & (((u >> np.uint64(sh)) & np.uint64(1)) == 1))
    u2 = u2 + np.where(up, np.uint64(1 << sh), np.uint64(0))
    return u2.astype(np.uint32).view(np.float32)


def cdiv(a, b):
    return (a + b - 1) // b


def chunks(total, step, start=0):
    return [(s, min(step, total - s)) for s in range(start, total, step)]


def build_graph():
    import concourse.bacc as bacc
    import concourse.mybir as mybir
    import concourse.tile as tile
    import contextlib

    F32, F32R = mybir.dt.float32, mybir.dt.float32r
    AF, OP = mybir.ActivationFunctionType, mybir.AluOpType

    M0 = SCHED[0][0]
    R0 = B_LOC * M0          # 1576
    NT0 = cdiv(R0, 128)      # 13
    SC = float(HD) ** -0.5

    nc = bacc.Bacc("TRN2", target_bir_lowering=False, debug=False, num_devices=N_CORES)

    xtok_d = nc.dram_tensor("xtok_fm", [D, R0], F32R, kind="ExternalInput")
    posrep_d = nc.dram_tensor("posrep", [R0, D], F32, kind="ExternalInput")
    pw_d = nc.dram_tensor("pw", [D, D], F32R, kind="ExternalInput")
    wq_d = nc.dram_tensor("wq", [L, D, D], F32R, kind="ExternalInput")
    wk_d = nc.dram_tensor("wk", [L, D, D], F32R, kind="ExternalInput")
    wv_d = nc.dram_tensor("wv", [L, D, D], F32R, kind="ExternalInput")
    wo_d = nc.dram_tensor("wo", [L, D, D], F32R, kind="ExternalInput")
    w1_d = nc.dram_tensor("w1", [L, D, FF], F32R, kind="ExternalInput")
    w2_d = nc.dram_tensor("w2", [L, FF, D], F32R, kind="ExternalInput")
    hw_d = nc.dram_tensor("hw", [D, V_OUT], F32R, kind="ExternalInput")
    iota_d = nc.dram_tensor("iota128f", [128, 128], F32, kind="ExternalInput")
    pcol_d = nc.dram_tensor("pcol128", [128, 1], F32, kind="ExternalInput")
    ident_d = nc.dram_tensor("ident128", [128, 128], F32R, kind="ExternalInput")
    e1r_d = nc.dram_tensor("e1r128", [128, 2], F32, kind="ExternalInput")
    onesf_d = nc.dram_tensor("onesf128", [128, 2], F32, kind="ExternalInput")
    onesr1_d = nc.dram_tensor("onesr_1x", [1, 128], F32R, kind="ExternalInput")
    ones1_d = nc.dram_tensor("ones_1x", [1, 128], F32, kind="ExternalInput")
    hmaskr_d = nc.dram_tensor("headmaskr", [D, 128], F32R, kind="ExternalInput")
    ones2dr_d = nc.dram_tensor("ones2dr", [128, 128], F32R, kind="ExternalInput")

    out_d = nc.dram_tensor("out", [B_LOC, V_OUT], F32, kind="ExternalOutput")

    with tile.TileContext(nc) as tc, contextlib.ExitStack() as ctx:
        ctx.enter_context(nc.allow_low_precision(reason="fp32r compute by design"))
        sb = ctx.enter_context(tc.tile_pool(name="sb", bufs=1))
        ps = ctx.enter_context(tc.tile_pool(name="ps", bufs=1, space="PSUM"))
        dram = ctx.enter_context(tc.tile_pool(name="dram", bufs=1, space="DRAM"))

        _uid = [0]

        def uid():
            _uid[0] += 1
            return _uid[0]

        def psA(p, f, name=None):
            return ps.tile([p, f], F32, tag="psA", name=f"pA{uid()}", bufs=4,
                           padded_shape=[128, 512])

        psQ = psA
        psI = psA

        def psB(p, f, name=None):
            return ps.tile([p, f], F32, tag="psB", name=f"pB{uid()}", bufs=2,
                           padded_shape=[128, 1024])

        # size-classed SBUF scratch: clsA = [128, <=864] f32r-ish slots
        def clsA(p, f, dtype, nbufs=8):
            return sb.tile([p, f], dtype, tag="clsA", name=f"cA{uid()}", bufs=nbufs,
                           padded_shape=[128, 768])

        def clsB(p, f, dtype, nbufs=18):
            return sb.tile([p, f], dtype, tag="clsB", name=f"cB{uid()}", bufs=nbufs,
                           padded_shape=[128, 640])

        # ---- consts
        iota_t = sb.tile([128, 128], F32, name="iota_t")
        pcol_t = sb.tile([128, 1], F32, name="pcol_t")
        nc.sync.dma_start(pcol_t[:], pcol_d[:, :])
        ident_t = sb.tile([128, 128], F32R, name="ident_t")
        e1r_t = sb.tile([128, 2], F32, name="e1r_t")
        onesf_t = sb.tile([128, 2], F32, name="onesf_t")
        onesr1_t = sb.tile([1, 128], F32R, name="onesr1_t")
        ones1_t = sb.tile([1, 128], F32, name="ones1_t")
        hmaskr_t = [sb.tile([128, 128], F32R, name=f"hmr{i}") for i in range(6)]
        ones2dr_t = sb.tile([128, 128], F32R, name="ones2dr_t")
        nc.sync.dma_start(ones2dr_t[:], ones2dr_d[:, :])
        nc.sync.dma_start(iota_t[:], iota_d[:, :])
        nc.sync.dma_start(ident_t[:], ident_d[:, :])
        nc.sync.dma_start(e1r_t[:], e1r_d[:, :])
        nc.sync.dma_start(onesf_t[:], onesf_d[:, :])
        nc.sync.dma_start(onesr1_t[:], onesr1_d[:, :])
        nc.sync.dma_start(ones1_t[:], ones1_d[:, :])
        for i in range(6):
            nc.sync.dma_start(hmaskr_t[i][:], hmaskr_d[128 * i:128 * (i + 1), :])
        eps_t = sb.tile([128, 1], F32, name="eps_t")
        nc.vector.memset(eps_t[:], LN_EPS)

        def rows_of(i, R):
            return min(128, R - 128 * i)

        # persistent residual tiles (13 x [128, 768] f32r)
        t_t = [sb.tile([128, D], F32R, tag=f"t{i}", name=f"t{i}") for i in range(NT0)]

        # weight slot: half-matrices [128, 3*768] (wq/wk/wv/wo halves), bufs=2
        def load_whalf(dram_ap, tag_l, half):
            t = sb.tile([128, 3 * D], F32R, tag="wslot", name=f"w{tag_l}_{half}_{uid()}",
                        bufs=6, padded_shape=[128, 3 * D])
            nc.sync.dma_start(
                t[:].rearrange("p (k n) -> p k n", k=3),
                dram_ap[128 * 3 * half:128 * 3 * (half + 1), :]
                .rearrange("(k p) n -> p k n", p=128))
            return t

        def ln_stats(tiles, R, nt_max):
            NT = cdiv(R, 128)
            sums = sb.tile([128, 2 * NT0], F32, tag="lnsums", name=f"sums{uid()}")
            junk = clsA(128, D, F32)
            for i in range(NT):
                r = rows_of(i, R)
                nc.scalar.activation(junk[:r, :], tiles[i][:r, :], AF.Identity,
                                     accum_out=sums[:r, i:i + 1])
                nc.scalar.activation(junk[:r, :], tiles[i][:r, :], AF.Square,
                                     accum_out=sums[:r, NT0 + i:NT0 + i + 1])
            stats = sb.tile([128, 4 * NT0], F32, tag="lnstats", name=f"stats{uid()}")
            mu = stats[:, 0:NT]
            rstd = stats[:, NT0:NT0 + NT]
            nmurs = stats[:, 2 * NT0:2 * NT0 + NT]
            scr = stats[:, 3 * NT0:3 * NT0 + NT]
            nc.vector.tensor_scalar(mu, sums[:, 0:NT], 1.0 / D, None, OP.mult)
            nc.vector.tensor_scalar(scr, sums[:, NT0:NT0 + NT], 1.0 / D, None, OP.mult)
            nc.vector.tensor_tensor(rstd, mu, mu, OP.mult)
            nc.vector.tensor_tensor(scr, scr, rstd, OP.subtract)      # var
            nc.scalar.activation(scr, scr, AF.Sqrt, bias=eps_t[:, :])  # std
            nc.vector.reciprocal(rstd, scr)
            nc.vector.tensor_tensor(nmurs, mu, rstd, OP.mult)
            nc.vector.tensor_scalar(nmurs, nmurs, -1.0, None, OP.mult)
            return stats

        def xn_cols(stats, c0, cw, R):
            """LN-applied xn feature-major covering cols [c0, c0+cw).
            Works on the 128-aligned covering range; returns (tiles, base)."""
            NT = cdiv(R, 128)
            rstd = stats[:, NT0:NT0 + NT]
            nmurs = stats[:, 2 * NT0:2 * NT0 + NT]
            t0a = (c0 // 128) * 128
            t1a = min(cdiv(c0 + cw, 128) * 128, ((R + 127) // 128) * 128)
            tis = list(range(t0a // 128, t1a // 128))
            cwa = sum(rows_of(ti, R) for ti in tis)
            dst = [clsB(128, cwa, F32R) for _ in range(6)]
            stage_list = []
            dl = 0
            for ti in tis:
                take = rows_of(ti, R)
                stage = clsA(128, D, F32R)
                nc.scalar.activation(stage[:take, :], t_t[ti][:take, :],
                                     AF.Identity,
                                     scale=rstd[:take, ti:ti + 1],
                                     bias=nmurs[:take, ti:ti + 1])
                stage_list.append((stage, take, dl))
                dl += take
            for kt in range(6):
                pt = psB(128, cwa)
                for j, (stage, take, dl) in enumerate(stage_list):
                    nc.tensor.matmul(
                        pt[:, dl:dl + take].bitcast(F32R),
                        stage[:take, 128 * kt:128 * (kt + 1)],
                        ident_t[:take, :take], is_transpose=True,
                        start=(j == 0), stop=(j == len(stage_list) - 1))
                nc.vector.tensor_copy(dst[kt][:, :cwa], pt[:, :cwa].bitcast(F32R))
            return dst, t0a

        def pair_cols(M):
            """[(w0, wlen, voffs per b in pair)] covering batches in pairs."""
            out = []
            for b0 in range(0, B_LOC, 2):
                out.append((b0 * M, 2 * M, b0))
            return out

        # =========================================================
        # patch embed: t = xtok @ pw + posrep
        # =========================================================
        pwh = [load_whalf(pw_d[:, :], "pw", h) for h in range(2)]
        for i in range(NT0):
            r = rows_of(i, R0)
            xtk = [clsB(128, 128, F32R) for _ in range(6)]
            for kt in range(6):
                nc.sync.dma_start(xtk[kt][:, :r],
                                  xtok_d[128 * kt:128 * (kt + 1), 128 * i:128 * i + r])
            pos_t = clsA(128, D, F32)
            nc.sync.dma_start(pos_t[:r, :], posrep_d[128 * i:128 * i + r, :])
            pt = psB(128, D)
            for n0, nw in chunks(D, 512):
                for kt in range(6):
                    half, k3 = kt // 3, kt % 3
                    nc.tensor.matmul(
                        pt[:, n0:n0 + nw],
                        xtk[kt][:, :],
                        pwh[half][:, D * k3 + n0:D * k3 + n0 + nw],
                        start=(kt == 0), stop=(kt == 5))
            nc.vector.tensor_tensor(t_t[i][:r, :], pt[:r, :], pos_t[:r, :], OP.add)

        # =========================================================
        # transformer layers
        # =========================================================
        o_dram = dram.tile([D, R0], F32, tag="odram", name="o_dram")
        for l in range(L):
            M, Mq, n_next = SCHED[l]
            R, Rq = B_LOC * M, B_LOC * Mq
            NT, NTq = cdiv(R, 128), cdiv(Rq, 128)

            st1 = ln_stats(t_t, R, NT)
            wkh = [load_whalf(wk_d[l], f"k{l}", h) for h in range(2)]
            wvh = [load_whalf(wv_d[l], f"v{l}", h) for h in range(2)]
            wqh = [load_whalf(wq_d[l], f"q{l}", h) for h in range(2)]

            den = sb.tile([12, 16], F32, tag="den", name=f"den{l}")
            qc_sb = sb.tile([128, 12], F32, tag="qc", name=f"qc{l}", bufs=1)
            qcbd = sb.tile([128, 128], F32R, tag="qcbd", name=f"qcbd{l}", bufs=1)
            imp_ps = psI(1, M)
            for pi, (w0, wl, b0) in enumerate(pair_cols(M)):
                xnp, xb = xn_cols(st1, w0, wl, R)
                xo = w0 - xb
                qc_ps = psQ(128, 12)
                for nt in range(6):
                    for kt in range(6):
                        half, k3 = kt // 3, kt % 3
                        nc.tensor.matmul(
                            qc_ps[:, 2 * nt:2 * nt + 2],
                            wqh[half][:, D * k3 + 128 * nt:D * k3 + 128 * (nt + 1)],
                            xnp[kt][:, xo:xo + wl:M],
                            start=(nt == 0 and kt == 0), stop=(nt == 5 and kt == 5))
                nc.scalar.activation(qc_sb[:], qc_ps[:], AF.Copy)
                kpre = [clsB(128, wl, F32R) for _ in range(6)]
                vpre = [clsB(128, wl, F32R) for _ in range(6)]
                for dst, wh in ((kpre, wkh), (vpre, wvh)):
                    for nt in range(6):
                        pt = psA(128, wl)
                        for kt in range(6):
                            h2, k32 = kt // 3, kt % 3
                            nc.tensor.matmul(
                                pt[:, :wl],
                                wh[h2][:, D * k32 + 128 * nt:D * k32 + 128 * (nt + 1)],
                                xnp[kt][:, xo:xo + wl],
                                start=(kt == 0), stop=(kt == 5))
                        nc.scalar.activation(dst[nt][:, :wl], pt[:, :wl], AF.Copy)
                # vnorm for the pair
                vp = psA(128, wl)
                sqv = clsB(128, wl, F32R)
                for kt in range(6):
                    nc.scalar.activation(sqv[:, :wl], vpre[kt][:, :wl], AF.Square)
                    nc.tensor.matmul(vp[:, :wl], hmaskr_t[kt][:], sqv[:, :wl],
                                     start=(kt == 0), stop=(kt == 5))
                vnp = clsB(12, wl, F32)
                nc.scalar.activation(vnp[:12, :wl], vp[:12, :wl], AF.Sqrt)
                # CLS attention -> importance contribution
                pclsp = clsB(12, wl, F32)
                for bi, b in enumerate((b0, b0 + 1)):
                    sc = psA(128, wl)
                    for kt in range(6):
                        nc.vector.tensor_scalar(qcbd[:], hmaskr_t[kt][:].bitcast(F32),
                                                qc_sb[:, 2 * kt + bi:2 * kt + bi + 1],
                                                None, OP.mult)
                        nc.tensor.matmul(sc[:, :wl], qcbd[:], kpre[kt][:, :wl],
                                         start=(kt == 0), stop=(kt == 5))
                    voff = bi * M
                    nc.scalar.activation(pclsp[:12, voff:voff + M],
                                         sc[:12, voff:voff + M], AF.Exp, scale=SC)
                    nc.vector.tensor_reduce(den[:, b:b + 1],
                                            pclsp[:12, voff:voff + M],
                                            mybir.AxisListType.X, OP.add)
                nc.vector.reciprocal(den[:, 8 + b0:10 + b0], den[:, b0:b0 + 2])
                for bi, b in enumerate((b0, b0 + 1)):
                    voff = bi * M
                    nc.vector.tensor_scalar(vnp[:12, voff:voff + M],
                                            vnp[:12, voff:voff + M],
                                            den[:, 8 + b:9 + b], None, OP.mult)
                nc.vector.tensor_tensor(pclsp[:12, :wl], pclsp[:12, :wl],
                                        vnp[:12, :wl], OP.mult)
                for bi, b in enumerate((b0, b0 + 1)):
                    nc.tensor.matmul(imp_ps[:, :], onesf_t[:12, 0:1],
                                     pclsp[:12, bi * M:bi * M + M],
                                     start=(b == 0), stop=(b == B_LOC - 1))
            # ---- AllReduce importance
            imp_sb = sb.tile([1, 200], F32, tag="imp", name=f"imp{l}")
            nc.vector.tensor_copy(imp_sb[:, :M], imp_ps[:, :])
            bin_t = dram.tile([1, M], F32, tag="arin", name=f"arin{l}")
            bout_t = dram.tile([1, M], F32, tag="arout", name=f"arout{l}",
                               addr_space="Shared")
            nc.sync.dma_start(bin_t[:], imp_sb[:, :M])
            nc.gpsimd.collective_compute(
                "AllReduce", OP.add, replica_groups=[list(range(N_CORES))],
                ins=[bin_t.opt()], outs=[bout_t.opt()])
            impg = sb.tile([1, 200], F32, tag="impg", name=f"impg{l}")
            nc.sync.dma_start(impg[:, :M], bout_t[:])

            # ---- ranks -> mask -> pos
            KC = cdiv(M, 128)
            impcol = sb.tile([128, 2], F32, tag="impcol", name=f"impcol{l}")
            for kc in range(KC):
                cnt = rows_of(kc, M)
                icp = psA(128, 1)
                nc.tensor.matmul(icp[:cnt, :], impg[:, 128 * kc:128 * kc + cnt],
                                 ones1_t[:, 0:1], start=True, stop=True)
                nc.vector.tensor_copy(impcol[:cnt, kc:kc + 1], icp[:cnt, :])
            rank_ps = psI(1, M)
            for kc in range(KC):
                cnt = rows_of(kc, M)
                ibc = psA(128, M)
                nc.tensor.matmul(ibc[:cnt, :], ones1_t[:, :cnt], impg[:, :M],
                                 start=True, stop=True)
                Ct = clsB(128, M, F32)
                eqt = clsB(128, M, F32)
                C2t = clsB(128, M, F32)
                nc.vector.tensor_scalar(Ct[:cnt, :M], ibc[:cnt, :M],
                                        impcol[:cnt, kc:kc + 1], None, OP.is_lt)
                nc.vector.tensor_scalar(eqt[:cnt, :M], ibc[:cnt, :M],
                                        impcol[:cnt, kc:kc + 1], None, OP.is_equal)
                # C2t mask: (global col j) > (global row k = 128*kc + p)
                pshk = sb.tile([128, 1], F32, tag="possh", name=f"pk{uid()}", bufs=4)
                for j0, jw in chunks(M, 128):
                    nc.vector.tensor_scalar(pshk[:cnt, :], pcol_t[:cnt, :],
                                            float(128 * kc - j0), None, OP.add)
                    nc.vector.tensor_scalar(C2t[:cnt, j0:j0 + jw],
                                            iota_t[:cnt, :jw], pshk[:cnt, :],
                                            None, OP.is_gt)
                nc.vector.tensor_tensor(C2t[:cnt, :M], eqt[:cnt, :M],
                                        C2t[:cnt, :M], OP.mult)
                lhs = e1r_t if kc == 0 else onesf_t
                nc.tensor.matmul(rank_ps[:, :], lhs[:cnt, 0:1], Ct[:cnt, :M],
                                 start=(kc == 0), stop=False)
                nc.tensor.matmul(rank_ps[:, :], lhs[:cnt, 0:1], C2t[:cnt, :M],
                                 start=False, stop=(kc == KC - 1))
            mask = sb.tile([1, 200], F32R, tag="imp", name=f"mask{l}")
            nc.vector.tensor_scalar(mask[:, :M], rank_ps[:, :], float(n_next), None,
                                    OP.is_lt)
            nc.vector.tensor_copy(mask[:, 0:1], onesf_t[0:1, 0:1])
            mflat = sb.tile([1, R0], F32R, tag="mflat", name=f"mflat{l}")
            for b in range(B_LOC):
                nc.vector.tensor_copy(mflat[:, b * M:(b + 1) * M], mask[:, :M])
            cum = sb.tile([1, R0], F32, tag="cum", name=f"cum{l}")
            nc.vector.tensor_tensor_scan(cum[:, :R], mflat[:, :R].bitcast(F32),
                                         mflat[:, :R].bitcast(F32), 0.0,
                                         OP.add, OP.max)
            nc.vector.tensor_tensor(cum[:, :R], cum[:, :R], mflat[:, :R].bitcast(F32),
                                    OP.subtract)  # exclusive positions, in place
            poscol = sb.tile([128, 2 * NT0], F32, tag="poscol", name=f"poscol{l}")
            for rc in range(NT):
                cnt = rows_of(rc, R)
                pcp = psA(128, 2)
                nc.tensor.matmul(pcp[:cnt, 0:1], cum[:, 128 * rc:128 * rc + cnt],
                                 ones1_t[:, 0:1], start=True, stop=False)
                nc.tensor.matmul(pcp[:cnt, 1:2],
                                 mflat[:, 128 * rc:128 * rc + cnt].bitcast(F32),
                                 ones1_t[:, 0:1], start=False, stop=True)
                nc.vector.tensor_copy(poscol[:cnt, 2 * rc:2 * rc + 2], pcp[:cnt, :])

            def win(rc):
                lo, hi = None, None
                for rr in range(128 * rc, min(128 * rc + 128, R)):
                    b, m = rr // M, rr % M
                    plo = b * Mq + max(0, m - (M - Mq))
                    phi = b * Mq + min(m, Mq - 1)
                    lo = plo if lo is None else min(lo, plo)
                    hi = phi if hi is None else max(hi, phi)
                return range(lo // 128, hi // 128 + 1)

            wins = [list(win(rc)) for rc in range(NT)]
            inv = [[rc for rc in range(NT) if cc in wins[rc]] for cc in range(NTq)]

            # ---- in-place gather of t (ascending cc; reads rc >= cc only)
            for cc in range(NTq):
                cq = rows_of(cc, Rq)
                rcs = inv[cc]
                stiles = []
                for rc in rcs:
                    cnt = rows_of(rc, R)
                    psh = sb.tile([128, 1], F32, tag="possh", name=f"ps{uid()}", bufs=4)
                    nc.vector.tensor_scalar(psh[:cnt, :],
                                            poscol[:cnt, 2 * rc:2 * rc + 1],
                                            -128.0 * cc, None, OP.add)
                    st = clsB(128, 128, F32R)
                    nc.vector.tensor_scalar(st[:cnt, :], iota_t[:cnt, :],
                                            psh[:cnt, :],
                                            poscol[:cnt, 2 * rc + 1:2 * rc + 2],
                                            OP.is_equal, OP.mult)
                    stiles.append((rc, cnt, st))
                gp = psB(128, D)
                for n0, nw in chunks(D, 512):
                    for ri, (rc, cnt, st) in enumerate(stiles):
                        nc.tensor.matmul(
                            gp[:, n0:n0 + nw], st[:cnt, :],
                            t_t[rc][:cnt, n0:n0 + nw],
                            start=(ri == 0), stop=(ri == len(stiles) - 1))
                nc.scalar.activation(t_t[cc][:cq, :], gp[:cq, :], AF.Copy)

            # ---- attention per pair on pruned tokens
            st2 = ln_stats(t_t, Rq, NTq)
            mkch = chunks(Mq, 128)
            Mq2 = Mq + (Mq % 2)  # even-padded free dim for fp32r matmuls
            NPV = max(Mq2, 256)  # extended moving width (1cyc fp32r)
            WPT = max(H * Mq2, (H - 1) * Mq2 + NPV,
                      ((H * Mq2 + 511) // 512 - 1) * 512 + 256 if H * Mq2 > 512 else 0)
            for (w0, wl, b0) in pair_cols(Mq):
                xnp, xb = xn_cols(st2, w0, wl, Rq)
                xo = w0 - xb
                qp = [clsB(128, 640, F32R) for _ in range(6)]
                kp = [clsB(128, 640, F32R) for _ in range(6)]
                for dst, wh in ((qp, wqh), (kp, wkh)):
                    for nt in range(6):
                        pt = psA(128, wl)
                        for kt in range(6):
                            h2, k32 = kt // 3, kt % 3
                            nc.tensor.matmul(
                                pt[:, :wl],
                                wh[h2][:, D * k32 + 128 * nt:D * k32 + 128 * (nt + 1)],
                                xnp[kt][:, xo:xo + wl],
                                start=(kt == 0), stop=(kt == 5))
                        nc.scalar.activation(dst[nt][:, :wl], pt[:, :wl], AF.Copy)
                vt = {}
                for bi, b in enumerate((b0, b0 + 1)):
                    for ci, (m0, mw) in enumerate(mkch):
                        v1 = clsA(128, D, F32R)
                        pt = psB(128, D)
                        for n0, nw in chunks(D, 512):
                            for kt in range(6):
                                h2, k32 = kt // 3, kt % 3
                                nc.tensor.matmul(
                                    pt[:mw, n0:n0 + nw],
                                    xnp[kt][:, xo + bi * Mq + m0:xo + bi * Mq + m0 + mw],
                                    wvh[h2][:, D * k32 + n0:D * k32 + n0 + nw],
                                    start=(kt == 0), stop=(kt == 5))
                        nc.scalar.activation(v1[:mw, :], pt[:mw, :], AF.Copy)
                        vt[(b, ci)] = v1
                op6 = [clsB(128, wl, F32R) for _ in range(6)]
                for bi, b in enumerate((b0, b0 + 1)):
                    boff = bi * Mq
                    ptall = [sb.tile([128, WPT], F32R, tag=f"pt{ci}",
                                     name=f"pt{uid()}",
                                     padded_shape=[128, 2112])
                             for ci in range(len(mkch))]
                    for h in range(12):
                        kt, ro = h // 2, 64 * (h % 2)
                        for ci, (m0, mw) in enumerate(mkch):
                            wlx = max(wl, 256)
                            sc = psA(128, wlx)
                            nc.tensor.matmul(
                                sc[:, :wlx],
                                kp[kt][ro:ro + 64, boff + m0:boff + m0 + 128],
                                qp[kt][ro:ro + 64, :wlx],
                                start=True, stop=True)
                            nc.scalar.activation(
                                ptall[ci][:mw, h * Mq2:h * Mq2 + Mq],
                                sc[:mw, boff:boff + Mq], AF.Exp, scale=SC)
                            if Mq2 != Mq:
                                nc.vector.memset(
                                    ptall[ci][:mw, h * Mq2 + Mq:(h + 1) * Mq2].bitcast(F32),
                                    0.0)
                    invd = sb.tile([1, WPT], F32R, tag="invd", name=f"iv{uid()}",
                                   padded_shape=[1, 2112], bufs=1)
                    for d0, dw in chunks(H * Mq2, 512):
                        dwx = max(dw, 256)
                        dn = psA(128, dwx)
                        for ci, (m0, mw) in enumerate(mkch):
                            nc.tensor.matmul(dn[:, :], ones2dr_t[:mw, :],
                                             ptall[ci][:mw, d0:d0 + dwx],
                                             start=(ci == 0),
                                             stop=(ci == len(mkch) - 1))
                        lntmp = clsB(1, dw, F32)
                        nc.scalar.activation(lntmp[:1, :dw], dn[0:1, :dw], AF.Ln)
                        nc.scalar.activation(invd[:, d0:d0 + dw], lntmp[:1, :dw],
                                             AF.Exp, scale=-1.0)
                    for h in range(12):
                        kt, ro = h // 2, 64 * (h % 2)
                        op_ = psA(128, NPV)
                        for ci, (m0, mw) in enumerate(mkch):
                            nc.tensor.matmul(op_[:, :],
                                             vt[(b, ci)][:mw, 128 * kt:128 * (kt + 1)],
                                             ptall[ci][:mw, h * Mq2:h * Mq2 + NPV],
                                             start=(ci == 0),
                                             stop=(ci == len(mkch) - 1))
                        ib = psA(128, NPV)
                        nc.tensor.matmul(ib[:, :], onesr1_t[:, :],
                                         invd[:, h * Mq2:h * Mq2 + NPV],
                                         start=True, stop=True)
                        nc.scalar.activation(op6[kt][ro:ro + 64, boff:boff + Mq],
                                             op_[ro:ro + 64, :Mq], AF.Copy)
                        nc.vector.tensor_tensor(op6[kt][ro:ro + 64, boff:boff + Mq],
                                                op6[kt][ro:ro + 64, boff:boff + Mq],
                                                ib[0:64, :Mq], OP.mult)
                for kt in range(6):
                    nc.sync.dma_start(o_dram[128 * kt:128 * (kt + 1), w0:w0 + wl],
                                      op6[kt][:, :wl].bitcast(F32))

            # ---- WO (streamed from o_dram) + residual
            woh = [load_whalf(wo_d[l], f"o{l}", h) for h in range(2)]
            for cc in range(NTq):
                cq = rows_of(cc, Rq)
                oc = [clsB(128, 128, F32R) for _ in range(6)]
                for kt in range(6):
                    nc.sync.dma_start(
                        oc[kt][:, :cq],
                        o_dram[128 * kt:128 * (kt + 1), 128 * cc:128 * cc + cq]
                        .bitcast(F32R))
                wp_ = psB(128, D)
                for n0, nw in chunks(D, 512):
                    for kt in range(6):
                        h2, k32 = kt // 3, kt % 3
                        nc.tensor.matmul(
                            wp_[:, n0:n0 + nw],
                            oc[kt][:, :],
                            woh[h2][:, D * k32 + n0:D * k32 + n0 + nw],
                            start=(kt == 0), stop=(kt == 5))
                nc.vector.tensor_tensor(t_t[cc][:cq, :], wp_[:cq, :],
                                        t_t[cc][:cq, :], OP.add)

            # ---- LN2 -> xn2 on the weight-slot ring; then MLP
            st3 = ln_stats(t_t, Rq, NTq)
            Rqp = cdiv(Rq, 256) * 256
            xn2 = [sb.tile([128, Rqp], F32R, tag="wslot", name=f"xn2_{l}_{kt}",
                           bufs=6, padded_shape=[128, 3 * D]) for kt in range(6)]
            for c0, cw in chunks(Rq, 512):
                sub, sb_ = xn_cols(st3, c0, cw, Rq)
                cwa = min(512, ((Rq + 127) // 128) * 128 - c0)
                cwv = min(cwa, Rq - c0)
                for kt in range(6):
                    nc.vector.tensor_copy(xn2[kt][:, c0:c0 + cwv], sub[kt][:, :cwv])
            GRP = 6
            for g0 in range(0, 24, GRP):
                gts = []
                for n1 in range(g0, g0 + GRP):
                    w1c = clsA(128, 6 * 128, F32R)
                    nc.sync.dma_start(
                        w1c[:].rearrange("p (k n) -> p k n", k=6),
                        w1_d[l, :, 128 * n1:128 * (n1 + 1)]
                        .rearrange("(k p) n -> p k n", p=128))
                    thirds = []
                    for c0, cw in chunks(Rq, 512):
                        cwx = max(cw, 256)
                        gt = clsB(128, 512, F32R)
                        pt = psA(128, cwx)
                        for kt in range(6):
                            nc.tensor.matmul(
                                pt[:, :cwx], w1c[:, 128 * kt:128 * (kt + 1)],
                                xn2[kt][:, c0:c0 + cwx],
                                start=(kt == 0), stop=(kt == 5))
                        nc.scalar.activation(gt[:, :cw], pt[:, :cw], AF.Gelu)
                        thirds.append((c0, gt))
                    gts.append(thirds)
                w2c = []
                for kt2 in range(g0, g0 + GRP):
                    wc = clsA(128, D, F32R)
                    nc.sync.dma_start(wc[:], w2_d[l, 128 * kt2:128 * (kt2 + 1), :])
                    w2c.append(wc)
                for cc in range(NTq):
                    cq = rows_of(cc, Rq)
                    wp2 = psB(128, D)
                    for n0, nw in chunks(D, 512):
                        for j in range(GRP):
                            c0, gsel = next(
                                (c0, g) for c0, g in gts[j]
                                if c0 <= 128 * cc < c0 + 512)
                            nc.tensor.matmul(
                                wp2[:, n0:n0 + nw],
                                gsel[:, 128 * cc - c0:128 * cc - c0 + 128],
                                w2c[j][:, n0:n0 + nw],
                                start=(j == 0), stop=(j == GRP - 1))
                    nc.vector.tensor_tensor(t_t[cc][:cq, :], wp2[:cq, :],
                                            t_t[cc][:cq, :], OP.add)

        # =========================================================
        # final LN + head on CLS rows
        # =========================================================
        Mf = SCHED[-1][1]
        Rf = B_LOC * Mf
        stf = ln_stats(t_t, Rf, cdiv(Rf, 128))
        rstdf = stf[:, NT0:NT0 + cdiv(Rf, 128)]
        nmursf = stf[:, 2 * NT0:2 * NT0 + cdiv(Rf, 128)]
        cls_raw = clsA(8, D, F32R)
        cls_st = sb.tile([8, 2], F32, tag="clsst", name="cls_st")
        for b in range(B_LOC):
            rr = b * Mf
            ti, off = rr // 128, rr % 128
            nc.sync.dma_start(cls_raw[b:b + 1, :], t_t[ti][off:off + 1, :])
            nc.sync.dma_start(cls_st[b:b + 1, 0:1], rstdf[off:off + 1, ti:ti + 1])
            nc.sync.dma_start(cls_st[b:b + 1, 1:2], nmursf[off:off + 1, ti:ti + 1])
        cls_tm = clsA(8, D, F32R)
        nc.scalar.activation(cls_tm[:8, :], cls_raw[:8, :], AF.Identity,
                             scale=cls_st[:8, 0:1], bias=cls_st[:8, 1:2])
        xcls_fm = [clsB(128, 128, F32R) for _ in range(6)]
        for kt in range(6):
            pt = psA(128, 8)
            nc.tensor.matmul(pt[:, 0:8].bitcast(F32R),
                             cls_tm[:8, 128 * kt:128 * (kt + 1)],
                             ident_t[:8, :8], is_transpose=True, start=True, stop=True)
            nc.vector.tensor_scalar(xcls_fm[kt][:, :], ones2dr_t[:].bitcast(F32), 0.0, None, OP.mult)
            nc.vector.tensor_copy(xcls_fm[kt][:, 0:8], pt[:, :].bitcast(F32R))
        out_sb = sb.tile([8, V_OUT], F32, tag="wslot", name="out_sb", bufs=6,
                         padded_shape=[128, 3 * D])
        for o0, ow in chunks(V_OUT, 512):
            hp = psA(128, ow)
            for kt in range(6):
                hwc = clsB(128, ow, F32R)
                nc.sync.dma_start(hwc[:, :ow], hw_d[128 * kt:128 * (kt + 1), o0:o0 + ow])
                nc.tensor.matmul(hp[:, :ow], xcls_fm[kt][:, :],
                                 hwc[:, :ow], start=(kt == 0), stop=(kt == 5))
            nc.scalar.activation(out_sb[:, o0:o0 + ow], hp[:8, :ow], AF.Copy)
        nc.sync.dma_start(out_d[:, :], out_sb[:])

    nc.compile()
    return nc


# =============================================================
# host side
# =============================================================
_CACHE = {}


def _consts():
    iota = np.broadcast_to(np.arange(128, dtype=np.float32), (128, 128)).copy()
    pcol = np.arange(128, dtype=np.float32).reshape(128, 1)
    ident = np.eye(128, dtype=np.float32)
    e1r = np.ones((128, 2), np.float32)
    e1r[0, :] = 0.0
    onesf = np.ones((128, 2), np.float32)
    onesr1 = np.ones((1, 128), np.float32)
    ones1 = np.ones((1, 128), np.float32)
    hmask = np.zeros((D, 128), np.float32)
    for d in range(D):
        hmask[d, (d % 128) // 64 + (d // 128) * 2] = 1.0
    ones2dr = np.ones((128, 128), np.float32)
    return dict(iota128f=iota, pcol128=pcol, ident128=ident,
                e1r128=e1r, onesf128=onesf, onesr_1x=onesr1, ones_1x=ones1,
                headmaskr=hmask, ones2dr=ones2dr)


def prepare_inputs(inputs):
    """Full inputs -> list of 8 per-core in_maps."""
    x = np.asarray(inputs["x"], np.float32)
    B = x.shape[0]
    M0 = SCHED[0][0]
    p = x.reshape(B, 3, GRID, PATCH, GRID, PATCH).transpose(0, 2, 4, 1, 3, 5)
    tokens = p.reshape(B, GRID * GRID, 3 * PATCH * PATCH)  # [B,196,768]
    pos = np.asarray(inputs["pos_emb"], np.float32).copy()  # [197,768]
    posf = pos.copy()
    posf[0] += np.asarray(inputs["cls_tok"], np.float32)
    posrep = np.tile(posf, (B_LOC, 1))  # [8*197, 768]

    w = {k: round_fp32r(np.asarray(inputs[k], np.float32))
         for k in ("patch_w", "wq", "wk", "wv", "wo", "w1", "w2", "head_w")}
    consts = _consts()
    consts["headmaskr"] = round_fp32r(consts["headmaskr"])

    in_maps = []
    for c in range(N_CORES):
        xt = np.zeros((B_LOC * M0, D), np.float32)
        for bl in range(B_LOC):
            xt[bl * M0 + 1:(bl + 1) * M0] = tokens[c * B_LOC + bl]
        m = dict(
            xtok_fm=np.ascontiguousarray(round_fp32r(xt).T),
            posrep=posrep,
            pw=w["patch_w"], wq=w["wq"], wk=w["wk"], wv=w["wv"], wo=w["wo"],
            w1=w["w1"], w2=w["w2"], hw=w["head_w"],
            **consts,
        )
        in_maps.append(m)
    return in_maps


def _get_runner():
    if "runner" not in _CACHE:
        import numpy as _np
        import jax
        from jax.sharding import Mesh, PartitionSpec, NamedSharding
        from jax.experimental.shard_map import shard_map
        import concourse.bass2jax as b2j
        import concourse.mybir as mybir

        nc = build_graph()
        b2j.install_neuronx_cc_hook()
        partition_name = nc.partition_id_tensor.name if nc.partition_id_tensor else None
        in_names, out_names, out_avals, zero_outs = [], [], [], []
        for alloc in nc.m.functions[0].allocations:
            if not isinstance(alloc, mybir.MemoryLocationSet):
                continue
            name = alloc.memorylocations[0].name
            if alloc.kind == "ExternalInput":
                if name != partition_name:
                    in_names.append(name)
            elif alloc.kind == "ExternalOutput":
                out_names.append(name)
                shape = tuple(alloc.tensor_shape)
                dtype = mybir.dt.np(alloc.dtype)
                out_avals.append(jax.core.ShapedArray(shape, dtype))
                zero_outs.append(_np.zeros(shape, dtype))
        n_params, n_outs = len(in_names), len(out_avals)
        all_in = list(in_names) + list(out_names)
        if partition_name:
            all_in.append(partition_name)

        def _body(*args):
            operands = list(args)
            if partition_name:
                operands.append(b2j.partition_id_tensor())
            return tuple(b2j._bass_exec_p.bind(
                *operands, out_avals=tuple(out_avals), in_names=tuple(all_in),
                out_names=tuple(out_names), lowering_input_output_aliases=(),
                sim_require_finite=True, sim_require_nnan=True, nc=nc))

        devices = jax.devices()[:N_CORES]
        mesh = Mesh(_np.asarray(devices), ("core",))
        sharded = jax.jit(
            shard_map(_body, mesh=mesh,
                      in_specs=(PartitionSpec("core"),) * (n_params + n_outs),
                      out_specs=(PartitionSpec("core"),) * n_outs,
                      check_rep=False),
            keep_unused=True)
        _CACHE["runner"] = (sharded, in_names, out_names, out_avals, zero_outs, mesh)
    return _CACHE["runner"]


def run_maps(in_maps):
    import jax
    import numpy as _np
    from jax.sharding import PartitionSpec, NamedSharding
    sharded, in_names, out_names, out_avals, zero_outs, mesh = _get_runner()
    per_core = [[_np.asarray(m[n]) for n in in_names] for m in in_maps]
    concat = [_np.concatenate([per_core[c][i] for c in range(N_CORES)], axis=0)
              for i in range(len(in_names))]
    concat += [_np.zeros((N_CORES * z.shape[0], *z.shape[1:]), z.dtype)
               for z in zero_outs]
    sh = NamedSharding(mesh, PartitionSpec("core"))
    handles = [jax.device_put(a, sh) for a in concat]
    outs = sharded(*handles)
    jax.block_until_ready(outs)
    return [
        {n: _np.asarray(outs[i]).reshape(N_CORES, *out_avals[i].shape)[c]
         for i, n in enumerate(out_names)}
        for c in range(N_CORES)
    ]


def kernel(**inputs):
    in_maps = prepare_inputs(inputs)
    res = run_maps(in_maps)
    return np.concatenate([res[c]["out"] for c in range(N_CORES)], axis=0)
